# revision 7
# baseline (speedup 1.0000x reference)
"""Distributed Trainium2 kernel for a dense transformer block.

Sharding (8 cores, one chip):
  - LN1/LN2 + FFN: sequence-parallel (each core owns 512 of the 4096 tokens).
  - Attention: head-parallel (each core owns 2 of the 16 heads).
  - Collectives: AllGather of ln1(x)^T (feature-major), AllToAll of per-head
    attention outputs back to token shards.  No AllReduce needed.
  - Matmuls run in float32r (full-rate reduced-precision fp32) with fp32
    accumulation in PSUM.
"""

import sys

sys.path.insert(0, "/opt/trn_rl_repo")

import numpy as np

import concourse.bacc as bacc
import concourse.bass as bass
import concourse.tile as tile
from concourse import mybir
from concourse.masks import make_identity

F32 = mybir.dt.float32
F32R = mybir.dt.float32r
AF = mybir.ActivationFunctionType

N_CORES = 8
B, T, D, H = 2, 2048, 1024, 16
HD = D // H            # 64
NTOK = B * T           # 4096
S = NTOK // N_CORES    # 512 tokens per core
HPC = H // N_CORES     # 2 heads per core
E = HPC * HD           # 128 head-dim columns per core
F = 4 * D              # 4096 ffn hidden
EPS = 1e-5
SCALE = float(D) ** -0.5
MASK_VAL = -30000.0
P = 128

KT = D // P            # 8 feature tiles
TT = S // P            # 4 token tiles in the shard
NW = N_CORES           # 8 global 512-token windows
ST_B = T // P          # 16 s-tiles per batch
FT = F // P            # 32 ffn-hidden tiles

_CACHE = {}


def _build():
    nc = bacc.Bacc("TRN2", target_bir_lowering=False, debug=False,
                   num_devices=N_CORES)

    x = nc.dram_tensor("x", [S, D], F32, kind="ExternalInput")
    wq = nc.dram_tensor("wq", [D, E], F32R, kind="ExternalInput")
    wk = nc.dram_tensor("wk", [D, E], F32R, kind="ExternalInput")
    wv = nc.dram_tensor("wv", [D, E], F32R, kind="ExternalInput")
    wo = nc.dram_tensor("wo", [D, D], F32R, kind="ExternalInput")
    w1 = nc.dram_tensor("w1", [D, F], F32R, kind="ExternalInput")
    w2 = nc.dram_tensor("w2", [F, D], F32R, kind="ExternalInput")
    bo = nc.dram_tensor("bo", [D], F32, kind="ExternalInput")
    b1 = nc.dram_tensor("b1", [F], F32, kind="ExternalInput")
    b2 = nc.dram_tensor("b2", [D], F32, kind="ExternalInput")
    ln1_g = nc.dram_tensor("ln1_g", [D], F32, kind="ExternalInput")
    ln1_b = nc.dram_tensor("ln1_b", [D], F32, kind="ExternalInput")
    ln2_g = nc.dram_tensor("ln2_g", [D], F32, kind="ExternalInput")
    ln2_b = nc.dram_tensor("ln2_b", [D], F32, kind="ExternalInput")
    y = nc.dram_tensor("y", [S, D], F32, kind="ExternalOutput")

    with tile.TileContext(nc) as tc:
        with tc.tile_pool(name="dram", bufs=1, space="DRAM") as dram, \
             tc.tile_pool(name="const", bufs=1) as const, \
             tc.tile_pool(name="persist", bufs=1) as persist:

            hT_sh = dram.tile([D, S], F32R, tag="hT_sh", name="hT_sh")
            hT_all = dram.tile([N_CORES * D, S], F32R, tag="hT_all",
                               name="hT_all", addr_space="Shared")
            a2a_in = dram.tile([NW * P, S], F32R, tag="a2a_in", name="a2a_in")
            a2a_out = dram.tile([NW * P, S], F32R, tag="a2a_out",
                                name="a2a_out")

            # ---- constants ----
            ident = const.tile([P, P], F32, tag="ident", name="ident")
            make_identity(nc, ident)

            ones_f = const.tile([P, HD], F32, tag="ones_f", name="ones_f")
            nc.vector.memset(ones_f[:], 1.0)
            ones_r = const.tile([P, HD], F32R, tag="ones_r", name="ones_r")
            nc.scalar.copy(ones_r[:], ones_f[:])

            eps_t = const.tile([P, 1], F32, tag="eps", name="eps_t")
            nc.vector.memset(eps_t[:], EPS)

            # ln params, feature-major [128, KT]
            g1_s = const.tile([P, KT], F32, tag="g1", name="g1_s")
            b1l_s = const.tile([P, KT], F32, tag="b1l", name="b1l_s")
            g2_s = const.tile([P, KT], F32, tag="g2", name="g2_s")
            b2l_s = const.tile([P, KT], F32, tag="b2l", name="b2l_s")
            nc.sync.dma_start(out=g1_s[:],
                              in_=ln1_g.ap().rearrange("(k p) -> p k", p=P))
            nc.sync.dma_start(out=b1l_s[:],
                              in_=ln1_b.ap().rearrange("(k p) -> p k", p=P))
            nc.sync.dma_start(out=g2_s[:],
                              in_=ln2_g.ap().rearrange("(k p) -> p k", p=P))
            nc.sync.dma_start(out=b2l_s[:],
                              in_=ln2_b.ap().rearrange("(k p) -> p k", p=P))

            # b1 (ffn bias), feature-major [128, FT]
            b1_s = const.tile([P, FT], F32, tag="b1s", name="b1_s")
            nc.sync.dma_start(out=b1_s[:],
                              in_=b1.ap().rearrange("(k p) -> p k", p=P))

            # bo, b2 broadcast across partitions [128, D]
            bo_bc = const.tile([P, D], F32, tag="bo_bc", name="bo_bc")
            b2_bc = const.tile([P, D], F32, tag="b2_bc", name="b2_bc")
            nc.sync.dma_start(out=bo_bc[:], in_=bo.ap().partition_broadcast(P))
            nc.sync.dma_start(out=b2_bc[:], in_=b2.ap().partition_broadcast(P))

            # causal masks for the 4 diagonal sub-positions: [128, 512] f32
            masks = []
            for k in range(4):
                m = const.tile([P, 512], F32, tag=f"mask{k}", name=f"mask{k}")
                nc.gpsimd.memset(m[:], 0.0)
                # keep 0 where t_local - p - 128k >= 0, else MASK_VAL
                nc.gpsimd.affine_select(
                    out=m[:], in_=m[:],
                    compare_op=mybir.AluOpType.is_ge,
                    fill=MASK_VAL, base=-128 * k,
                    pattern=[[1, 512]], channel_multiplier=-1,
                )
                masks.append(m)

            # persistent across most of the kernel: x shard, r1, h2T
            x_sb = [persist.tile([P, D], F32, tag=f"x{i}", name=f"x{i}")
                    for i in range(TT)]
            for i in range(TT):
                nc.sync.dma_start(out=x_sb[i][:], in_=x[i * P:(i + 1) * P, :])
            r1 = [persist.tile([P, D], F32, tag=f"r1_{i}", name=f"r1_{i}")
                  for i in range(TT)]
            h2T = [persist.tile([P, S], F32R, tag=f"h2T{k}", name=f"h2T{k}")
                   for k in range(KT)]

            def layernorm_tiles(src_tiles, pool, out_name):
                """LN over the free axis -> normalized [t,d] f32 tiles.
                gamma/beta are folded in at transpose-evict time."""
                out = []
                with tc.tile_pool(name=f"ln_{out_name}", bufs=2) as lnp:
                    for i, xt in enumerate(src_tiles):
                        st = lnp.tile([P, 2, 6], F32, tag="stats", name="st")
                        xr = xt[:].rearrange("p (s f) -> p s f", s=2)
                        for sg in range(2):
                            nc.vector.bn_stats(out=st[:, sg, :], in_=xr[:, sg, :])
                        mv = lnp.tile([P, 2], F32, tag="mv", name="mv")
                        nc.vector.bn_aggr(out=mv[:], in_=st[:])
                        rstd = lnp.tile([P, 1], F32, tag="rstd", name="rstd")
                        nc.scalar.activation(out=rstd[:], in_=mv[:, 1:2],
                                             func=AF.Sqrt, bias=eps_t[:],
                                             scale=1.0)
                        nc.vector.reciprocal(out=rstd[:], in_=rstd[:])
                        o = pool.tile([P, D], F32, tag=f"{out_name}{i}",
                                      name=f"{out_name}{i}")
                        nc.vector.tensor_scalar(
                            out=o[:], in0=xt[:],
                            scalar1=mv[:, 0:1], scalar2=rstd[:],
                            op0=mybir.AluOpType.subtract,
                            op1=mybir.AluOpType.mult,
                        )
                        out.append(o)
                return out

            # ============ attention super-phase (scoped SBUF) ============
            with tc.tile_pool(name="attnsb", bufs=1) as attnsb:
                # -------- LN1 + transpose + AllGather --------
                with tc.tile_pool(name="xlnp", bufs=1) as xlnp:
                    xln = layernorm_tiles(x_sb, xlnp, "xln")
                    with tc.tile_pool(name="tr1", bufs=3) as trp, \
                         tc.tile_pool(name="tr1p", bufs=3, space="PSUM") as trpp:
                        for i in range(TT):
                            for kt in range(KT):
                                pt = trpp.tile([P, P], F32, tag="tr", name="pt")
                                nc.tensor.transpose(
                                    pt[:], xln[i][:, kt * P:(kt + 1) * P],
                                    ident[:])
                                hb = trp.tile([P, P], F32R, tag="hb", name="hb")
                                nc.vector.tensor_scalar(
                                    out=hb[:], in0=pt[:],
                                    scalar1=g1_s[:, kt:kt + 1],
                                    scalar2=b1l_s[:, kt:kt + 1],
                                    op0=mybir.AluOpType.mult,
                                    op1=mybir.AluOpType.add,
                                )
                                nc.sync.dma_start(
                                    out=hT_sh[kt * P:(kt + 1) * P,
                                              i * P:(i + 1) * P],
                                    in_=hb[:])

                nc.gpsimd.collective_compute(
                    "AllGather", mybir.AluOpType.bypass,
                    replica_groups=[list(range(N_CORES))],
                    ins=[hT_sh.opt()], outs=[hT_all.opt()],
                )

                # -------- QKV projections --------
                wq_sb = [attnsb.tile([P, E], F32R, tag=f"wq{k}", name=f"wq{k}")
                         for k in range(KT)]
                wk_sb = [attnsb.tile([P, E], F32R, tag=f"wk{k}", name=f"wk{k}")
                         for k in range(KT)]
                wv_sb = [attnsb.tile([P, E], F32R, tag=f"wv{k}", name=f"wv{k}")
                         for k in range(KT)]
                for k in range(KT):
                    nc.sync.dma_start(out=wq_sb[k][:],
                                      in_=wq[k * P:(k + 1) * P, :])
                    nc.sync.dma_start(out=wk_sb[k][:],
                                      in_=wk[k * P:(k + 1) * P, :])
                    nc.sync.dma_start(out=wv_sb[k][:],
                                      in_=wv[k * P:(k + 1) * P, :])

                qT = attnsb.tile([P, NTOK], F32R, tag="qT", name="qT")
                kTt = attnsb.tile([P, NTOK], F32R, tag="kT", name="kTt")
                v_sb = [attnsb.tile([P, E], F32R, tag=f"v{s}", name=f"v{s}")
                        for s in range(NTOK // P)]

                with tc.tile_pool(name="hstream", bufs=4) as hsp, \
                     tc.tile_pool(name="vtmp", bufs=2) as vtp, \
                     tc.tile_pool(name="qkvp", bufs=2, space="PSUM") as qkvp, \
                     tc.tile_pool(name="vtrp", bufs=2, space="PSUM") as vtrp:
                    for tch in range(NW):
                        psq = qkvp.tile([P, 512], F32, tag="psq", name="psq")
                        psk = qkvp.tile([P, 512], F32, tag="psk", name="psk")
                        psv = qkvp.tile([P, 512], F32, tag="psv", name="psv")
                        for kt in range(KT):
                            ht = hsp.tile([P, 512], F32R, tag="ht", name="ht")
                            nc.sync.dma_start(
                                out=ht[:],
                                in_=hT_all[tch * D + kt * P:
                                           tch * D + (kt + 1) * P, :])
                            first, last = kt == 0, kt == KT - 1
                            nc.tensor.matmul(psq[:], wq_sb[kt][:], ht[:],
                                             start=first, stop=last)
                            nc.tensor.matmul(psk[:], wk_sb[kt][:], ht[:],
                                             start=first, stop=last)
                            nc.tensor.matmul(psv[:], wv_sb[kt][:], ht[:],
                                             start=first, stop=last)
                        nc.scalar.copy(qT[:, tch * 512:(tch + 1) * 512], psq[:])
                        nc.scalar.copy(kTt[:, tch * 512:(tch + 1) * 512],
                                       psk[:])
                        vt = vtp.tile([P, 512], F32, tag="vt", name="vt")
                        nc.scalar.copy(vt[:], psv[:])
                        for j in range(4):
                            pv = vtrp.tile([P, P], F32, tag="pv", name="pv")
                            nc.tensor.transpose(pv[:], vt[:, j * P:(j + 1) * P],
                                                ident[:])
                            nc.vector.tensor_copy(v_sb[tch * 4 + j][:], pv[:])

                # -------- attention --------
                with tc.tile_pool(name="pt_pool", bufs=4) as ptp, \
                     tc.tile_pool(name="attno", bufs=2) as aop, \
                     tc.tile_pool(name="scp", bufs=2, space="PSUM") as scp, \
                     tc.tile_pool(name="lop", bufs=1, space="PSUM") as lop:
                    for b in range(B):
                        for tcl in range(T // 512):
                            tch = b * (T // 512) + tcl
                            l_psa = lop.tile([HD, 512], F32, tag="la", name="l_psa")
                            l_psb = lop.tile([HD, 512], F32, tag="lb", name="l_psb")
                            o_psa = lop.tile([HD, 512], F32, tag="oa", name="o_psa")
                            o_psb = lop.tile([HD, 512], F32, tag="ob", name="o_psb")
                            n_s = 4 * (tcl + 1)
                            for si in range(n_s):
                                sg = b * ST_B + si
                                sc_a = scp.tile([P, 512], F32, tag="sca",
                                                name="sc_a")
                                sc_b = scp.tile([P, 512], F32, tag="scb",
                                                name="sc_b")
                                nc.tensor.matmul(
                                    sc_a[:], kTt[0:HD, sg * P:(sg + 1) * P],
                                    qT[0:HD, tch * 512:(tch + 1) * 512],
                                    start=True, stop=True,
                                    tile_position=(0, 0))
                                nc.tensor.matmul(
                                    sc_b[:], kTt[HD:P, sg * P:(sg + 1) * P],
                                    qT[HD:P, tch * 512:(tch + 1) * 512],
                                    start=True, stop=True,
                                    tile_position=(64, 0))
                                if si // 4 == tcl:
                                    mk = masks[si % 4]
                                    nc.vector.tensor_add(out=sc_a[:],
                                                         in0=sc_a[:],
                                                         in1=mk[:])
                                    nc.vector.tensor_add(out=sc_b[:],
                                                         in0=sc_b[:],
                                                         in1=mk[:])
                                p_a = ptp.tile([P, 512], F32R, tag="pa",
                                               name="p_a")
                                p_b = ptp.tile([P, 512], F32R, tag="pb",
                                               name="p_b")
                                nc.scalar.activation(out=p_a[:], in_=sc_a[:],
                                                     func=AF.Exp, scale=SCALE)
                                nc.scalar.activation(out=p_b[:], in_=sc_b[:],
                                                     func=AF.Exp, scale=SCALE)
                                first, last = si == 0, si == n_s - 1
                                nc.tensor.matmul(l_psa[:], ones_r[:, 0:HD],
                                                 p_a[:], start=first, stop=last)
                                nc.tensor.matmul(l_psb[:], ones_r[:, 0:HD],
                                                 p_b[:], start=first, stop=last)
                                nc.tensor.matmul(o_psa[:],
                                                 v_sb[sg][:, 0:HD], p_a[:],
                                                 start=first, stop=last)
                                nc.tensor.matmul(o_psb[:],
                                                 v_sb[sg][:, HD:E], p_b[:],
                                                 start=first, stop=last)
                            linv = aop.tile([P, 512], F32, tag="linv",
                                            name="linv")
                            nc.vector.reciprocal(out=linv[0:HD, :], in_=l_psa[:])
                            nc.vector.reciprocal(out=linv[HD:P, :], in_=l_psb[:])
                            o_n = aop.tile([P, 512], F32R, tag="on", name="o_n")
                            nc.vector.tensor_mul(out=o_n[0:HD, :], in0=o_psa[:],
                                                 in1=linv[0:HD, :])
                            nc.vector.tensor_mul(out=o_n[HD:P, :], in0=o_psb[:],
                                                 in1=linv[HD:P, :])
                            nc.sync.dma_start(
                                out=a2a_in[tch * P:(tch + 1) * P, :],
                                in_=o_n[:])

                nc.gpsimd.collective_compute(
                    "AllToAll", mybir.AluOpType.bypass,
                    replica_groups=[list(range(N_CORES))],
                    ins=[a2a_in.opt()], outs=[a2a_out.opt()],
                )
            # attnsb closed: qT/kT/v/wqkv SBUF freed

            # -------- output projection + residual --------
            with tc.tile_pool(name="wos", bufs=3) as wos, \
                 tc.tile_pool(name="aos", bufs=3) as aos, \
                 tc.tile_pool(name="wop", bufs=1, space="PSUM") as wop:
                pso = [wop.tile([P, 512], F32, tag=f"wo{i}", name=f"wo{i}")
                       for i in range(8)]
                for kt in range(KT):
                    ao = aos.tile([P, S], F32R, tag="ao", name="ao")
                    nc.sync.dma_start(out=ao[:],
                                      in_=a2a_out[kt * P:(kt + 1) * P, :])
                    wot = wos.tile([P, D], F32R, tag="wot", name="wot")
                    nc.sync.dma_start(out=wot[:],
                                      in_=wo[kt * P:(kt + 1) * P, :])
                    first, last = kt == 0, kt == KT - 1
                    for tt in range(TT):
                        for dc in range(2):
                            nc.tensor.matmul(
                                pso[tt * 2 + dc][:],
                                ao[:, tt * P:(tt + 1) * P],
                                wot[:, dc * 512:(dc + 1) * 512],
                                start=first, stop=last)
                for tt in range(TT):
                    for dc in range(2):
                        sl = slice(dc * 512, (dc + 1) * 512)
                        nc.vector.tensor_add(out=r1[tt][:, sl],
                                             in0=pso[tt * 2 + dc][:],
                                             in1=x_sb[tt][:, sl])
                        nc.vector.tensor_add(out=r1[tt][:, sl],
                                             in0=r1[tt][:, sl],
                                             in1=bo_bc[:, sl])

            # -------- LN2 + transpose --------
            with tc.tile_pool(name="h2p", bufs=1) as h2p:
                h2 = layernorm_tiles(r1, h2p, "h2")
                with tc.tile_pool(name="tr2p", bufs=3, space="PSUM") as tr2p:
                    for i in range(TT):
                        for kt in range(KT):
                            pt2 = tr2p.tile([P, P], F32, tag="tr2", name="pt2")
                            nc.tensor.transpose(
                                pt2[:], h2[i][:, kt * P:(kt + 1) * P], ident[:])
                            nc.vector.tensor_scalar(
                                out=h2T[kt][:, i * P:(i + 1) * P], in0=pt2[:],
                                scalar1=g2_s[:, kt:kt + 1],
                                scalar2=b2l_s[:, kt:kt + 1],
                                op0=mybir.AluOpType.mult,
                                op1=mybir.AluOpType.add,
                            )

            # -------- FFN --------
            with tc.tile_pool(name="ff1sb", bufs=1) as ff1sb:
                ff1 = [ff1sb.tile([P, S], F32R, tag=f"ff1_{k}",
                                  name=f"ff1_{k}") for k in range(FT)]
                with tc.tile_pool(name="w1s", bufs=4) as w1s, \
                     tc.tile_pool(name="ff1p", bufs=3, space="PSUM") as ff1p:
                    for ft in range(FT):
                        ps = ff1p.tile([P, S], F32, tag="ff1", name="ps")
                        for kt in range(KT):
                            w1t = w1s.tile([P, P], F32R, tag="w1t", name="w1t")
                            nc.sync.dma_start(
                                out=w1t[:],
                                in_=w1[kt * P:(kt + 1) * P,
                                       ft * P:(ft + 1) * P])
                            nc.tensor.matmul(ps[:], w1t[:], h2T[kt][:],
                                             start=(kt == 0),
                                             stop=(kt == KT - 1))
                        nc.scalar.activation(out=ff1[ft][:], in_=ps[:],
                                             func=AF.Relu,
                                             bias=b1_s[:, ft:ft + 1])

                with tc.tile_pool(name="w2s", bufs=3) as w2s, \
                     tc.tile_pool(name="outp", bufs=2) as outp, \
                     tc.tile_pool(name="ff2p", bufs=1, space="PSUM") as ff2p:
                    ps2 = [ff2p.tile([P, 512], F32, tag=f"ff2_{i}",
                                     name=f"ff2_{i}") for i in range(8)]
                    for kt in range(FT):
                        w2t = w2s.tile([P, D], F32R, tag="w2t", name="w2t")
                        nc.sync.dma_start(out=w2t[:],
                                          in_=w2[kt * P:(kt + 1) * P, :])
                        first, last = kt == 0, kt == FT - 1
                        for tt in range(TT):
                            for dc in range(2):
                                nc.tensor.matmul(
                                    ps2[tt * 2 + dc][:],
                                    ff1[kt][:, tt * P:(tt + 1) * P],
                                    w2t[:, dc * 512:(dc + 1) * 512],
                                    start=first, stop=last)
                    for tt in range(TT):
                        for dc in range(2):
                            sl = slice(dc * 512, (dc + 1) * 512)
                            ot = outp.tile([P, 512], F32, tag="ot", name="ot")
                            nc.vector.tensor_add(out=ot[:],
                                                 in0=ps2[tt * 2 + dc][:],
                                                 in1=r1[tt][:, sl])
                            nc.vector.tensor_add(out=ot[:], in0=ot[:],
                                                 in1=b2_bc[:, sl])
                            nc.sync.dma_start(out=y[tt * P:(tt + 1) * P, sl],
                                              in_=ot[:])

    nc.compile()
    return nc


def _shard_inputs(inputs):
    x = np.ascontiguousarray(np.asarray(inputs["x"], np.float32).reshape(NTOK, D))
    Wq = np.asarray(inputs["Wq"], np.float32)
    Wk = np.asarray(inputs["Wk"], np.float32)
    Wv = np.asarray(inputs["Wv"], np.float32)
    com = dict(
        wo=np.ascontiguousarray(np.asarray(inputs["Wo"], np.float32)),
        w1=np.ascontiguousarray(np.asarray(inputs["W1"], np.float32)),
        w2=np.ascontiguousarray(np.asarray(inputs["W2"], np.float32)),
        bo=np.asarray(inputs["bo"], np.float32),
        b1=np.asarray(inputs["b1"], np.float32),
        b2=np.asarray(inputs["b2"], np.float32),
        ln1_g=np.asarray(inputs["ln1_g"], np.float32),
        ln1_b=np.asarray(inputs["ln1_b"], np.float32),
        ln2_g=np.asarray(inputs["ln2_g"], np.float32),
        ln2_b=np.asarray(inputs["ln2_b"], np.float32),
    )
    maps = []
    for c in range(N_CORES):
        hs = slice(HPC * c, HPC * (c + 1))
        m = dict(com)
        m["x"] = x[c * S:(c + 1) * S]
        m["wq"] = np.ascontiguousarray(Wq[hs].transpose(1, 0, 2).reshape(D, E))
        m["wk"] = np.ascontiguousarray(Wk[hs].transpose(1, 0, 2).reshape(D, E))
        m["wv"] = np.ascontiguousarray(Wv[hs].transpose(1, 0, 2).reshape(D, E))
        maps.append(m)
    return maps


def _get_nc():
    if "nc" not in _CACHE:
        _CACHE["nc"] = _build()
    return _CACHE["nc"]


def _run(in_maps):
    from concourse.bass_utils import run_bass_kernel_spmd
    nc = _get_nc()
    res = run_bass_kernel_spmd(nc, in_maps, core_ids=list(range(N_CORES)))
    return res.results


def kernel(**inputs):
    in_maps = _shard_inputs(inputs)
    results = _run(in_maps)
    out = np.concatenate([results[c]["y"] for c in range(N_CORES)], axis=0)
    return out.reshape(B, T, D)


# revision 8
# speedup vs baseline: 426.9726x; 426.9726x over previous
"""Distributed Trainium2 kernel for a dense transformer block.

Sharding (8 cores, one chip):
  - LN1/LN2 + FFN: sequence-parallel (each core owns 512 of the 4096 tokens).
  - Attention: head-parallel (each core owns 2 of the 16 heads).
  - Collectives: AllGather of ln1(x)^T (feature-major), AllToAll of per-head
    attention outputs back to token shards.  No AllReduce needed.
  - Matmuls run in float32r (full-rate reduced-precision fp32) with fp32
    accumulation in PSUM.
"""

import sys

sys.path.insert(0, "/opt/trn_rl_repo")

import numpy as np

import concourse.bacc as bacc
import concourse.bass as bass
import concourse.tile as tile
from concourse import mybir
from concourse.masks import make_identity

F32 = mybir.dt.float32
F32R = mybir.dt.float32r
AF = mybir.ActivationFunctionType

N_CORES = 8
B, T, D, H = 2, 2048, 1024, 16
HD = D // H            # 64
NTOK = B * T           # 4096
S = NTOK // N_CORES    # 512 tokens per core
HPC = H // N_CORES     # 2 heads per core
E = HPC * HD           # 128 head-dim columns per core
F = 4 * D              # 4096 ffn hidden
EPS = 1e-5
SCALE = float(D) ** -0.5
MASK_VAL = -30000.0
P = 128

KT = D // P            # 8 feature tiles
TT = S // P            # 4 token tiles in the shard
NW = N_CORES           # 8 global 512-token windows
ST_B = T // P          # 16 s-tiles per batch
FT = F // P            # 32 ffn-hidden tiles

_CACHE = {}


def _build(n_chain=1):
    nc = bacc.Bacc("TRN2", target_bir_lowering=False, debug=False,
                   num_devices=N_CORES)

    x = nc.dram_tensor("x", [S, D], F32, kind="ExternalInput")
    wq = nc.dram_tensor("wq", [D, E], F32R, kind="ExternalInput")
    wk = nc.dram_tensor("wk", [D, E], F32R, kind="ExternalInput")
    wv = nc.dram_tensor("wv", [D, E], F32R, kind="ExternalInput")
    wo = nc.dram_tensor("wo", [D, D], F32R, kind="ExternalInput")
    w1 = nc.dram_tensor("w1", [D, F], F32R, kind="ExternalInput")
    w2 = nc.dram_tensor("w2", [F, D], F32R, kind="ExternalInput")
    bo = nc.dram_tensor("bo", [D], F32, kind="ExternalInput")
    b1 = nc.dram_tensor("b1", [F], F32, kind="ExternalInput")
    b2 = nc.dram_tensor("b2", [D], F32, kind="ExternalInput")
    ln1_g = nc.dram_tensor("ln1_g", [D], F32, kind="ExternalInput")
    ln1_b = nc.dram_tensor("ln1_b", [D], F32, kind="ExternalInput")
    ln2_g = nc.dram_tensor("ln2_g", [D], F32, kind="ExternalInput")
    ln2_b = nc.dram_tensor("ln2_b", [D], F32, kind="ExternalInput")
    y = nc.dram_tensor("y", [S, D], F32, kind="ExternalOutput")
    global _W
    _W = dict(wq=wq, wk=wk, wv=wv, wo=wo, w1=w1, w2=w2, bo=bo, b1=b1, b2=b2,
              ln1_g=ln1_g, ln1_b=ln1_b, ln2_g=ln2_g, ln2_b=ln2_b)

    with tile.TileContext(nc) as tc:
      with tc.tile_pool(name="dram0", bufs=1, space="DRAM") as dram0:
        chain_bufs = [dram0.tile([S, D], F32, tag=f"chain{i}", name=f"chain{i}")
                      for i in range(n_chain - 1)]
        for _ci in range(n_chain):
            x_cur = x if _ci == 0 else chain_bufs[_ci - 1]
            y_cur = y if _ci == n_chain - 1 else chain_bufs[_ci]
            _emit_body(nc, tc, x_cur, y_cur, _ci)

    nc.compile()
    return nc


def _emit_body(nc, tc, x, y, ci):
    wq, wk, wv, wo = _W["wq"], _W["wk"], _W["wv"], _W["wo"]
    w1, w2, bo, b1, b2 = _W["w1"], _W["w2"], _W["bo"], _W["b1"], _W["b2"]
    ln1_g, ln1_b = _W["ln1_g"], _W["ln1_b"]
    ln2_g, ln2_b = _W["ln2_g"], _W["ln2_b"]
    with 1 == 1 and tc.tile_pool(name=f"body{ci}", bufs=1) as _unused:
        with tc.tile_pool(name="dram", bufs=1, space="DRAM") as dram, \
             tc.tile_pool(name="const", bufs=1) as const, \
             tc.tile_pool(name="persist", bufs=1) as persist:

            hT_sh = dram.tile([D, S], F32R, tag="hT_sh", name="hT_sh")
            hT_all = dram.tile([N_CORES * D, S], F32R, tag="hT_all",
                               name="hT_all", addr_space="Shared")
            a2a_in = dram.tile([NW * P, S], F32R, tag="a2a_in", name="a2a_in")
            a2a_out = dram.tile([NW * P, S], F32R, tag="a2a_out",
                                name="a2a_out")

            # ---- constants ----
            ident = const.tile([P, P], F32, tag="ident", name="ident")
            make_identity(nc, ident)

            ones_f = const.tile([P, HD], F32, tag="ones_f", name="ones_f")
            nc.vector.memset(ones_f[:], 1.0)
            ones_r = const.tile([P, HD], F32R, tag="ones_r", name="ones_r")
            nc.scalar.copy(ones_r[:], ones_f[:])

            eps_t = const.tile([P, 1], F32, tag="eps", name="eps_t")
            nc.vector.memset(eps_t[:], EPS)

            # ln params, feature-major [128, KT]
            g1_s = const.tile([P, KT], F32, tag="g1", name="g1_s")
            b1l_s = const.tile([P, KT], F32, tag="b1l", name="b1l_s")
            g2_s = const.tile([P, KT], F32, tag="g2", name="g2_s")
            b2l_s = const.tile([P, KT], F32, tag="b2l", name="b2l_s")
            nc.sync.dma_start(out=g1_s[:],
                              in_=ln1_g.ap().rearrange("(k p) -> p k", p=P))
            nc.sync.dma_start(out=b1l_s[:],
                              in_=ln1_b.ap().rearrange("(k p) -> p k", p=P))
            nc.sync.dma_start(out=g2_s[:],
                              in_=ln2_g.ap().rearrange("(k p) -> p k", p=P))
            nc.sync.dma_start(out=b2l_s[:],
                              in_=ln2_b.ap().rearrange("(k p) -> p k", p=P))

            # b1 (ffn bias), feature-major [128, FT]
            b1_s = const.tile([P, FT], F32, tag="b1s", name="b1_s")
            nc.sync.dma_start(out=b1_s[:],
                              in_=b1.ap().rearrange("(k p) -> p k", p=P))

            # bo, b2 broadcast across partitions [128, D]
            bo_bc = const.tile([P, D], F32, tag="bo_bc", name="bo_bc")
            b2_bc = const.tile([P, D], F32, tag="b2_bc", name="b2_bc")
            nc.sync.dma_start(out=bo_bc[:], in_=bo.ap().partition_broadcast(P))
            nc.sync.dma_start(out=b2_bc[:], in_=b2.ap().partition_broadcast(P))

            # causal masks for the 4 diagonal sub-positions: [128, 512] f32
            masks = []
            for k in range(4):
                m = const.tile([P, 512], F32, tag=f"mask{k}", name=f"mask{k}")
                nc.gpsimd.memset(m[:], 0.0)
                # keep 0 where t_local - p - 128k >= 0, else MASK_VAL
                nc.gpsimd.affine_select(
                    out=m[:], in_=m[:],
                    compare_op=mybir.AluOpType.is_ge,
                    fill=MASK_VAL, base=-128 * k,
                    pattern=[[1, 512]], channel_multiplier=-1,
                )
                masks.append(m)

            # persistent across most of the kernel: x shard, r1, h2T
            x_sb = [persist.tile([P, D], F32, tag=f"x{i}", name=f"x{i}")
                    for i in range(TT)]
            for i in range(TT):
                nc.sync.dma_start(out=x_sb[i][:], in_=x[i * P:(i + 1) * P, :])
            r1 = [persist.tile([P, D], F32, tag=f"r1_{i}", name=f"r1_{i}")
                  for i in range(TT)]
            h2T = [persist.tile([P, S], F32R, tag=f"h2T{k}", name=f"h2T{k}")
                   for k in range(KT)]

            def layernorm_tiles(src_tiles, pool, out_name):
                """LN over the free axis -> normalized [t,d] f32 tiles.
                gamma/beta are folded in at transpose-evict time."""
                out = []
                with tc.tile_pool(name=f"ln_{out_name}", bufs=2) as lnp:
                    for i, xt in enumerate(src_tiles):
                        st = lnp.tile([P, 2, 6], F32, tag="stats", name="st")
                        xr = xt[:].rearrange("p (s f) -> p s f", s=2)
                        for sg in range(2):
                            nc.vector.bn_stats(out=st[:, sg, :], in_=xr[:, sg, :])
                        mv = lnp.tile([P, 2], F32, tag="mv", name="mv")
                        nc.vector.bn_aggr(out=mv[:], in_=st[:])
                        rstd = lnp.tile([P, 1], F32, tag="rstd", name="rstd")
                        nc.scalar.activation(out=rstd[:], in_=mv[:, 1:2],
                                             func=AF.Sqrt, bias=eps_t[:],
                                             scale=1.0)
                        nc.vector.reciprocal(out=rstd[:], in_=rstd[:])
                        o = pool.tile([P, D], F32, tag=f"{out_name}{i}",
                                      name=f"{out_name}{i}")
                        nc.vector.tensor_scalar(
                            out=o[:], in0=xt[:],
                            scalar1=mv[:, 0:1], scalar2=rstd[:],
                            op0=mybir.AluOpType.subtract,
                            op1=mybir.AluOpType.mult,
                        )
                        out.append(o)
                return out

            # ============ attention super-phase (scoped SBUF) ============
            with tc.tile_pool(name="attnsb", bufs=1) as attnsb:
                # -------- LN1 + transpose + AllGather --------
                with tc.tile_pool(name="xlnp", bufs=1) as xlnp:
                    xln = layernorm_tiles(x_sb, xlnp, "xln")
                    with tc.tile_pool(name="tr1", bufs=3) as trp, \
                         tc.tile_pool(name="tr1p", bufs=3, space="PSUM") as trpp:
                        for i in range(TT):
                            for kt in range(KT):
                                pt = trpp.tile([P, P], F32, tag="tr", name="pt")
                                nc.tensor.transpose(
                                    pt[:], xln[i][:, kt * P:(kt + 1) * P],
                                    ident[:])
                                hb = trp.tile([P, P], F32R, tag="hb", name="hb")
                                nc.vector.tensor_scalar(
                                    out=hb[:], in0=pt[:],
                                    scalar1=g1_s[:, kt:kt + 1],
                                    scalar2=b1l_s[:, kt:kt + 1],
                                    op0=mybir.AluOpType.mult,
                                    op1=mybir.AluOpType.add,
                                )
                                nc.sync.dma_start(
                                    out=hT_sh[kt * P:(kt + 1) * P,
                                              i * P:(i + 1) * P],
                                    in_=hb[:])

                nc.gpsimd.collective_compute(
                    "AllGather", mybir.AluOpType.bypass,
                    replica_groups=[list(range(N_CORES))],
                    ins=[hT_sh.opt()], outs=[hT_all.opt()],
                )

                # -------- QKV projections --------
                wq_sb = [attnsb.tile([P, E], F32R, tag=f"wq{k}", name=f"wq{k}")
                         for k in range(KT)]
                wk_sb = [attnsb.tile([P, E], F32R, tag=f"wk{k}", name=f"wk{k}")
                         for k in range(KT)]
                wv_sb = [attnsb.tile([P, E], F32R, tag=f"wv{k}", name=f"wv{k}")
                         for k in range(KT)]
                for k in range(KT):
                    nc.sync.dma_start(out=wq_sb[k][:],
                                      in_=wq[k * P:(k + 1) * P, :])
                    nc.sync.dma_start(out=wk_sb[k][:],
                                      in_=wk[k * P:(k + 1) * P, :])
                    nc.sync.dma_start(out=wv_sb[k][:],
                                      in_=wv[k * P:(k + 1) * P, :])

                qT = attnsb.tile([P, NTOK], F32R, tag="qT", name="qT")
                kTt = attnsb.tile([P, NTOK], F32R, tag="kT", name="kTt")
                v_sb = [attnsb.tile([P, E], F32R, tag=f"v{s}", name=f"v{s}")
                        for s in range(NTOK // P)]

                with tc.tile_pool(name="hstream", bufs=4) as hsp, \
                     tc.tile_pool(name="vtmp", bufs=2) as vtp, \
                     tc.tile_pool(name="qkvp", bufs=2, space="PSUM") as qkvp, \
                     tc.tile_pool(name="vtrp", bufs=2, space="PSUM") as vtrp:
                    for tch in range(NW):
                        psq = qkvp.tile([P, 512], F32, tag="psq", name="psq")
                        psk = qkvp.tile([P, 512], F32, tag="psk", name="psk")
                        psv = qkvp.tile([P, 512], F32, tag="psv", name="psv")
                        for kt in range(KT):
                            ht = hsp.tile([P, 512], F32R, tag="ht", name="ht")
                            nc.sync.dma_start(
                                out=ht[:],
                                in_=hT_all[tch * D + kt * P:
                                           tch * D + (kt + 1) * P, :])
                            first, last = kt == 0, kt == KT - 1
                            nc.tensor.matmul(psq[:], wq_sb[kt][:], ht[:],
                                             start=first, stop=last)
                            nc.tensor.matmul(psk[:], wk_sb[kt][:], ht[:],
                                             start=first, stop=last)
                            nc.tensor.matmul(psv[:], wv_sb[kt][:], ht[:],
                                             start=first, stop=last)
                        nc.scalar.copy(qT[:, tch * 512:(tch + 1) * 512], psq[:])
                        nc.scalar.copy(kTt[:, tch * 512:(tch + 1) * 512],
                                       psk[:])
                        vt = vtp.tile([P, 512], F32, tag="vt", name="vt")
                        nc.scalar.copy(vt[:], psv[:])
                        for j in range(4):
                            pv = vtrp.tile([P, P], F32, tag="pv", name="pv")
                            nc.tensor.transpose(pv[:], vt[:, j * P:(j + 1) * P],
                                                ident[:])
                            nc.vector.tensor_copy(v_sb[tch * 4 + j][:], pv[:])

                # -------- attention --------
                with tc.tile_pool(name="pt_pool", bufs=4) as ptp, \
                     tc.tile_pool(name="attno", bufs=2) as aop, \
                     tc.tile_pool(name="scp", bufs=2, space="PSUM") as scp, \
                     tc.tile_pool(name="lop", bufs=1, space="PSUM") as lop:
                    for b in range(B):
                        for tcl in range(T // 512):
                            tch = b * (T // 512) + tcl
                            l_psa = lop.tile([HD, 512], F32, tag="la", name="l_psa")
                            l_psb = lop.tile([HD, 512], F32, tag="lb", name="l_psb")
                            o_psa = lop.tile([HD, 512], F32, tag="oa", name="o_psa")
                            o_psb = lop.tile([HD, 512], F32, tag="ob", name="o_psb")
                            n_s = 4 * (tcl + 1)
                            for si in range(n_s):
                                sg = b * ST_B + si
                                sc_a = scp.tile([P, 512], F32, tag="sca",
                                                name="sc_a")
                                sc_b = scp.tile([P, 512], F32, tag="scb",
                                                name="sc_b")
                                nc.tensor.matmul(
                                    sc_a[:], kTt[0:HD, sg * P:(sg + 1) * P],
                                    qT[0:HD, tch * 512:(tch + 1) * 512],
                                    start=True, stop=True,
                                    tile_position=(0, 0))
                                nc.tensor.matmul(
                                    sc_b[:], kTt[HD:P, sg * P:(sg + 1) * P],
                                    qT[HD:P, tch * 512:(tch + 1) * 512],
                                    start=True, stop=True,
                                    tile_position=(64, 0))
                                if si // 4 == tcl:
                                    mk = masks[si % 4]
                                    nc.vector.tensor_add(out=sc_a[:],
                                                         in0=sc_a[:],
                                                         in1=mk[:])
                                    nc.vector.tensor_add(out=sc_b[:],
                                                         in0=sc_b[:],
                                                         in1=mk[:])
                                p_a = ptp.tile([P, 512], F32R, tag="pa",
                                               name="p_a")
                                p_b = ptp.tile([P, 512], F32R, tag="pb",
                                               name="p_b")
                                nc.scalar.activation(out=p_a[:], in_=sc_a[:],
                                                     func=AF.Exp, scale=SCALE)
                                nc.scalar.activation(out=p_b[:], in_=sc_b[:],
                                                     func=AF.Exp, scale=SCALE)
                                first, last = si == 0, si == n_s - 1
                                nc.tensor.matmul(l_psa[:], ones_r[:, 0:HD],
                                                 p_a[:], start=first, stop=last)
                                nc.tensor.matmul(l_psb[:], ones_r[:, 0:HD],
                                                 p_b[:], start=first, stop=last)
                                nc.tensor.matmul(o_psa[:],
                                                 v_sb[sg][:, 0:HD], p_a[:],
                                                 start=first, stop=last)
                                nc.tensor.matmul(o_psb[:],
                                                 v_sb[sg][:, HD:E], p_b[:],
                                                 start=first, stop=last)
                            linv = aop.tile([P, 512], F32, tag="linv",
                                            name="linv")
                            nc.vector.reciprocal(out=linv[0:HD, :], in_=l_psa[:])
                            nc.vector.reciprocal(out=linv[HD:P, :], in_=l_psb[:])
                            o_n = aop.tile([P, 512], F32R, tag="on", name="o_n")
                            nc.vector.tensor_mul(out=o_n[0:HD, :], in0=o_psa[:],
                                                 in1=linv[0:HD, :])
                            nc.vector.tensor_mul(out=o_n[HD:P, :], in0=o_psb[:],
                                                 in1=linv[HD:P, :])
                            nc.sync.dma_start(
                                out=a2a_in[tch * P:(tch + 1) * P, :],
                                in_=o_n[:])

                nc.gpsimd.collective_compute(
                    "AllToAll", mybir.AluOpType.bypass,
                    replica_groups=[list(range(N_CORES))],
                    ins=[a2a_in.opt()], outs=[a2a_out.opt()],
                )
            # attnsb closed: qT/kT/v/wqkv SBUF freed

            # -------- output projection + residual --------
            with tc.tile_pool(name="wos", bufs=3) as wos, \
                 tc.tile_pool(name="aos", bufs=3) as aos, \
                 tc.tile_pool(name="wop", bufs=1, space="PSUM") as wop:
                pso = [wop.tile([P, 512], F32, tag=f"wo{i}", name=f"wo{i}")
                       for i in range(8)]
                for kt in range(KT):
                    ao = aos.tile([P, S], F32R, tag="ao", name="ao")
                    nc.sync.dma_start(out=ao[:],
                                      in_=a2a_out[kt * P:(kt + 1) * P, :])
                    wot = wos.tile([P, D], F32R, tag="wot", name="wot")
                    nc.sync.dma_start(out=wot[:],
                                      in_=wo[kt * P:(kt + 1) * P, :])
                    first, last = kt == 0, kt == KT - 1
                    for tt in range(TT):
                        for dc in range(2):
                            nc.tensor.matmul(
                                pso[tt * 2 + dc][:],
                                ao[:, tt * P:(tt + 1) * P],
                                wot[:, dc * 512:(dc + 1) * 512],
                                start=first, stop=last)
                for tt in range(TT):
                    for dc in range(2):
                        sl = slice(dc * 512, (dc + 1) * 512)
                        nc.vector.tensor_add(out=r1[tt][:, sl],
                                             in0=pso[tt * 2 + dc][:],
                                             in1=x_sb[tt][:, sl])
                        nc.vector.tensor_add(out=r1[tt][:, sl],
                                             in0=r1[tt][:, sl],
                                             in1=bo_bc[:, sl])

            # -------- LN2 + transpose --------
            with tc.tile_pool(name="h2p", bufs=1) as h2p:
                h2 = layernorm_tiles(r1, h2p, "h2")
                with tc.tile_pool(name="tr2p", bufs=3, space="PSUM") as tr2p:
                    for i in range(TT):
                        for kt in range(KT):
                            pt2 = tr2p.tile([P, P], F32, tag="tr2", name="pt2")
                            nc.tensor.transpose(
                                pt2[:], h2[i][:, kt * P:(kt + 1) * P], ident[:])
                            nc.vector.tensor_scalar(
                                out=h2T[kt][:, i * P:(i + 1) * P], in0=pt2[:],
                                scalar1=g2_s[:, kt:kt + 1],
                                scalar2=b2l_s[:, kt:kt + 1],
                                op0=mybir.AluOpType.mult,
                                op1=mybir.AluOpType.add,
                            )

            # -------- FFN --------
            with tc.tile_pool(name="ff1sb", bufs=1) as ff1sb:
                ff1 = [ff1sb.tile([P, S], F32R, tag=f"ff1_{k}",
                                  name=f"ff1_{k}") for k in range(FT)]
                with tc.tile_pool(name="w1s", bufs=4) as w1s, \
                     tc.tile_pool(name="ff1p", bufs=3, space="PSUM") as ff1p:
                    for ft in range(FT):
                        ps = ff1p.tile([P, S], F32, tag="ff1", name="ps")
                        for kt in range(KT):
                            w1t = w1s.tile([P, P], F32R, tag="w1t", name="w1t")
                            nc.sync.dma_start(
                                out=w1t[:],
                                in_=w1[kt * P:(kt + 1) * P,
                                       ft * P:(ft + 1) * P])
                            nc.tensor.matmul(ps[:], w1t[:], h2T[kt][:],
                                             start=(kt == 0),
                                             stop=(kt == KT - 1))
                        nc.scalar.activation(out=ff1[ft][:], in_=ps[:],
                                             func=AF.Relu,
                                             bias=b1_s[:, ft:ft + 1])

                with tc.tile_pool(name="w2s", bufs=3) as w2s, \
                     tc.tile_pool(name="outp", bufs=2) as outp, \
                     tc.tile_pool(name="ff2p", bufs=1, space="PSUM") as ff2p:
                    ps2 = [ff2p.tile([P, 512], F32, tag=f"ff2_{i}",
                                     name=f"ff2_{i}") for i in range(8)]
                    for kt in range(FT):
                        w2t = w2s.tile([P, D], F32R, tag="w2t", name="w2t")
                        nc.sync.dma_start(out=w2t[:],
                                          in_=w2[kt * P:(kt + 1) * P, :])
                        first, last = kt == 0, kt == FT - 1
                        for tt in range(TT):
                            for dc in range(2):
                                nc.tensor.matmul(
                                    ps2[tt * 2 + dc][:],
                                    ff1[kt][:, tt * P:(tt + 1) * P],
                                    w2t[:, dc * 512:(dc + 1) * 512],
                                    start=first, stop=last)
                    for tt in range(TT):
                        for dc in range(2):
                            sl = slice(dc * 512, (dc + 1) * 512)
                            ot = outp.tile([P, 512], F32, tag="ot", name="ot")
                            nc.vector.tensor_add(out=ot[:],
                                                 in0=ps2[tt * 2 + dc][:],
                                                 in1=r1[tt][:, sl])
                            nc.vector.tensor_add(out=ot[:], in0=ot[:],
                                                 in1=b2_bc[:, sl])
                            nc.sync.dma_start(out=y[tt * P:(tt + 1) * P, sl],
                                              in_=ot[:])


def _shard_inputs(inputs):
    x = np.ascontiguousarray(np.asarray(inputs["x"], np.float32).reshape(NTOK, D))
    Wq = np.asarray(inputs["Wq"], np.float32)
    Wk = np.asarray(inputs["Wk"], np.float32)
    Wv = np.asarray(inputs["Wv"], np.float32)
    com = dict(
        wo=np.ascontiguousarray(np.asarray(inputs["Wo"], np.float32)),
        w1=np.ascontiguousarray(np.asarray(inputs["W1"], np.float32)),
        w2=np.ascontiguousarray(np.asarray(inputs["W2"], np.float32)),
        bo=np.asarray(inputs["bo"], np.float32),
        b1=np.asarray(inputs["b1"], np.float32),
        b2=np.asarray(inputs["b2"], np.float32),
        ln1_g=np.asarray(inputs["ln1_g"], np.float32),
        ln1_b=np.asarray(inputs["ln1_b"], np.float32),
        ln2_g=np.asarray(inputs["ln2_g"], np.float32),
        ln2_b=np.asarray(inputs["ln2_b"], np.float32),
    )
    maps = []
    for c in range(N_CORES):
        hs = slice(HPC * c, HPC * (c + 1))
        m = dict(com)
        m["x"] = x[c * S:(c + 1) * S]
        m["wq"] = np.ascontiguousarray(Wq[hs].transpose(1, 0, 2).reshape(D, E))
        m["wk"] = np.ascontiguousarray(Wk[hs].transpose(1, 0, 2).reshape(D, E))
        m["wv"] = np.ascontiguousarray(Wv[hs].transpose(1, 0, 2).reshape(D, E))
        maps.append(m)
    return maps


def _get_nc():
    if "nc" not in _CACHE:
        _CACHE["nc"] = _build()
    return _CACHE["nc"]


def _run(in_maps):
    from concourse.bass_utils import run_bass_kernel_spmd
    nc = _get_nc()
    res = run_bass_kernel_spmd(nc, in_maps, core_ids=list(range(N_CORES)))
    return res.results


def kernel(**inputs):
    in_maps = _shard_inputs(inputs)
    results = _run(in_maps)
    out = np.concatenate([results[c]["y"] for c in range(N_CORES)], axis=0)
    return out.reshape(B, T, D)


# revision 13
# speedup vs baseline: 5311.6405x; 12.4402x over previous
"""Distributed Trainium2 kernel for a dense transformer block.

Sharding (8 cores, one chip):
  - LN1/LN2 + FFN: sequence-parallel (each core owns 512 of the 4096 tokens).
  - Attention: head-parallel (each core owns 2 of the 16 heads).
  - Collectives: AllGather of ln1(x)^T (feature-major), AllToAll of per-head
    attention outputs back to token shards.  No AllReduce needed.
  - Matmuls run in float32r (full-rate reduced-precision fp32) with fp32
    accumulation in PSUM.
"""

import sys

sys.path.insert(0, "/opt/trn_rl_repo")

import numpy as np

import concourse.bacc as bacc
import concourse.bass as bass
import concourse.tile as tile
from concourse import mybir
from concourse.masks import make_identity

F32 = mybir.dt.float32
F32R = mybir.dt.float32r
BF16 = mybir.dt.bfloat16
AF = mybir.ActivationFunctionType

N_CORES = 8
B, T, D, H = 2, 2048, 1024, 16
HD = D // H            # 64
NTOK = B * T           # 4096
S = NTOK // N_CORES    # 512 tokens per core
HPC = H // N_CORES     # 2 heads per core
E = HPC * HD           # 128 head-dim columns per core
F = 4 * D              # 4096 ffn hidden
EPS = 1e-5
SCALE = float(D) ** -0.5
MASK_VAL = -30000.0
P = 128

KT = D // P            # 8 feature tiles
TT = S // P            # 4 token tiles in the shard
NW = N_CORES           # 8 global 512-token windows
ST_B = T // P          # 16 s-tiles per batch
FT = F // P            # 32 ffn-hidden tiles

_CACHE = {}


def _build(n_chain=1, stub_cc=False, upto=9):
    nc = bacc.Bacc("TRN2", target_bir_lowering=False, debug=False,
                   num_devices=N_CORES)

    x = nc.dram_tensor("x", [S, D], F32, kind="ExternalInput")
    wq = nc.dram_tensor("wq", [D, E], BF16, kind="ExternalInput")
    wk = nc.dram_tensor("wk", [D, E], BF16, kind="ExternalInput")
    wv = nc.dram_tensor("wv", [D, E], BF16, kind="ExternalInput")
    wo = nc.dram_tensor("wo", [D, D], F32R, kind="ExternalInput")
    w1 = nc.dram_tensor("w1", [D, F], F32R, kind="ExternalInput")
    w2 = nc.dram_tensor("w2", [F, D], F32R, kind="ExternalInput")
    bo = nc.dram_tensor("bo", [D], F32, kind="ExternalInput")
    b1 = nc.dram_tensor("b1", [F], F32, kind="ExternalInput")
    b2 = nc.dram_tensor("b2", [D], F32, kind="ExternalInput")
    ln1_g = nc.dram_tensor("ln1_g", [D], F32, kind="ExternalInput")
    ln1_b = nc.dram_tensor("ln1_b", [D], F32, kind="ExternalInput")
    ln2_g = nc.dram_tensor("ln2_g", [D], F32, kind="ExternalInput")
    ln2_b = nc.dram_tensor("ln2_b", [D], F32, kind="ExternalInput")
    y = nc.dram_tensor("y", [S, D], F32, kind="ExternalOutput")
    global _W
    _W = dict(wq=wq, wk=wk, wv=wv, wo=wo, w1=w1, w2=w2, bo=bo, b1=b1, b2=b2,
              ln1_g=ln1_g, ln1_b=ln1_b, ln2_g=ln2_g, ln2_b=ln2_b)

    with tile.TileContext(nc) as tc:
      with tc.tile_pool(name="dram0", bufs=1, space="DRAM") as dram0:
        chain_bufs = [dram0.tile([S, D], F32, tag=f"chain{i}", name=f"chain{i}")
                      for i in range(n_chain - 1)]
        for _ci in range(n_chain):
            x_cur = x if _ci == 0 else chain_bufs[_ci - 1]
            y_cur = y if _ci == n_chain - 1 else chain_bufs[_ci]
            _emit_body(nc, tc, x_cur, y_cur, _ci, stub_cc, upto)

    nc.compile()
    return nc


def _emit_body(nc, tc, x, y, ci, stub_cc=False, upto=9):
    wq, wk, wv, wo = _W["wq"], _W["wk"], _W["wv"], _W["wo"]
    w1, w2, bo, b1, b2 = _W["w1"], _W["w2"], _W["bo"], _W["b1"], _W["b2"]
    ln1_g, ln1_b = _W["ln1_g"], _W["ln1_b"]
    ln2_g, ln2_b = _W["ln2_g"], _W["ln2_b"]
    with 1 == 1 and tc.tile_pool(name=f"body{ci}", bufs=1) as _unused:
        with tc.tile_pool(name="dram", bufs=1, space="DRAM") as dram, \
             tc.tile_pool(name="const", bufs=1) as const, \
             tc.tile_pool(name="persist", bufs=1) as persist:

            hT_sh = dram.tile([D, S], BF16, tag="hT_sh", name="hT_sh")
            hT_all = dram.tile([N_CORES * D, S], BF16, tag="hT_all",
                               name="hT_all", addr_space="Shared")
            a2a_in = dram.tile([NW * P, S], F32R, tag="a2a_in", name="a2a_in")
            a2a_out = dram.tile([NW * P, S], F32R, tag="a2a_out",
                                name="a2a_out")

            # ---- constants ----
            ident = const.tile([P, P], F32, tag="ident", name="ident")
            make_identity(nc, ident)

            ones_r = const.tile([P, HD], BF16, tag="ones_r", name="ones_r")
            nc.vector.memset(ones_r[:], 1.0)

            eps_t = const.tile([P, 1], F32, tag="eps", name="eps_t")
            nc.vector.memset(eps_t[:], EPS)

            # ln params, feature-major [128, KT]
            g1_s = const.tile([P, KT], F32, tag="g1", name="g1_s")
            b1l_s = const.tile([P, KT], F32, tag="b1l", name="b1l_s")
            g2_s = const.tile([P, KT], F32, tag="g2", name="g2_s")
            b2l_s = const.tile([P, KT], F32, tag="b2l", name="b2l_s")
            nc.sync.dma_start(out=g1_s[:],
                              in_=ln1_g.ap().rearrange("(k p) -> p k", p=P))
            nc.sync.dma_start(out=b1l_s[:],
                              in_=ln1_b.ap().rearrange("(k p) -> p k", p=P))
            nc.sync.dma_start(out=g2_s[:],
                              in_=ln2_g.ap().rearrange("(k p) -> p k", p=P))
            nc.sync.dma_start(out=b2l_s[:],
                              in_=ln2_b.ap().rearrange("(k p) -> p k", p=P))

            # b1 (ffn bias), feature-major [128, FT]
            b1_s = const.tile([P, FT], F32, tag="b1s", name="b1_s")
            nc.sync.dma_start(out=b1_s[:],
                              in_=b1.ap().rearrange("(k p) -> p k", p=P))

            # bo, b2 broadcast across partitions [128, D]
            bo_bc = const.tile([P, D], F32, tag="bo_bc", name="bo_bc")
            b2_bc = const.tile([P, D], F32, tag="b2_bc", name="b2_bc")
            nc.sync.dma_start(out=bo_bc[:], in_=bo.ap().partition_broadcast(P))
            nc.sync.dma_start(out=b2_bc[:], in_=b2.ap().partition_broadcast(P))

            # persistent across most of the kernel: x shard, r1, h2T
            x_sb = [persist.tile([P, D], F32, tag=f"x{i}", name=f"x{i}")
                    for i in range(TT)]
            for i in range(TT):
                nc.sync.dma_start(out=x_sb[i][:], in_=x[i * P:(i + 1) * P, :])
            r1 = [persist.tile([P, D], F32, tag=f"r1_{i}", name=f"r1_{i}")
                  for i in range(TT)]
            h2T = [persist.tile([P, S], F32R, tag=f"h2T{k}", name=f"h2T{k}")
                   for k in range(KT)]

            def layernorm_tiles(src_tiles, pool, out_name):
                """LN over the free axis -> normalized [t,d] f32 tiles.
                gamma/beta are folded in at transpose-evict time."""
                out = []
                with tc.tile_pool(name=f"ln_{out_name}", bufs=2) as lnp:
                    for i, xt in enumerate(src_tiles):
                        st = lnp.tile([P, 2, 6], F32, tag="stats", name="st")
                        xr = xt[:].rearrange("p (s f) -> p s f", s=2)
                        for sg in range(2):
                            nc.vector.bn_stats(out=st[:, sg, :], in_=xr[:, sg, :])
                        mv = lnp.tile([P, 2], F32, tag="mv", name="mv")
                        nc.vector.bn_aggr(out=mv[:], in_=st[:])
                        rstd = lnp.tile([P, 1], F32, tag="rstd", name="rstd")
                        nc.scalar.activation(out=rstd[:], in_=mv[:, 1:2],
                                             func=AF.Sqrt, bias=eps_t[:],
                                             scale=1.0)
                        nc.vector.reciprocal(out=rstd[:], in_=rstd[:])
                        o = pool.tile([P, D], F32, tag=f"{out_name}{i}",
                                      name=f"{out_name}{i}")
                        nc.vector.tensor_scalar(
                            out=o[:], in0=xt[:],
                            scalar1=mv[:, 0:1], scalar2=rstd[:],
                            op0=mybir.AluOpType.subtract,
                            op1=mybir.AluOpType.mult,
                        )
                        out.append(o)
                return out

            # ============ attention super-phase (scoped SBUF) ============
            with tc.tile_pool(name="attnsb", bufs=1) as attnsb:
                # -------- LN1 + transpose + AllGather --------
                with tc.tile_pool(name="xlnp", bufs=1) as xlnp:
                    xln = layernorm_tiles(x_sb, xlnp, "xln")
                    with tc.tile_pool(name="tr1", bufs=4) as trp, \
                         tc.tile_pool(name="tr1p", bufs=4, space="PSUM") as trpp:
                        for kt in range(KT):
                            hb = trp.tile([P, S], BF16, tag="hb", name="hb")
                            for i in range(TT):
                                pt = trpp.tile([P, P], F32, tag="tr", name="pt")
                                nc.tensor.transpose(
                                    pt[:], xln[i][:, kt * P:(kt + 1) * P],
                                    ident[:])
                                nc.vector.tensor_scalar(
                                    out=hb[:, i * P:(i + 1) * P], in0=pt[:],
                                    scalar1=g1_s[:, kt:kt + 1],
                                    scalar2=b1l_s[:, kt:kt + 1],
                                    op0=mybir.AluOpType.mult,
                                    op1=mybir.AluOpType.add,
                                )
                            nc.sync.dma_start(
                                out=hT_sh[kt * P:(kt + 1) * P, :], in_=hb[:])

                if stub_cc:
                    nc.sync.dma_start(out=hT_all[0:D, :], in_=hT_sh[:, :])
                else:
                    nc.gpsimd.collective_compute(
                        "AllGather", mybir.AluOpType.bypass,
                        replica_groups=[list(range(N_CORES))],
                        ins=[hT_sh.opt()], outs=[hT_all.opt()],
                    )

                # -------- QKV projections --------
                if upto < 2:
                    return
                wq_sb = [attnsb.tile([P, E], BF16, tag=f"wq{k}", name=f"wq{k}")
                         for k in range(KT)]
                wk_sb = [attnsb.tile([P, E], BF16, tag=f"wk{k}", name=f"wk{k}")
                         for k in range(KT)]
                wv_sb = [attnsb.tile([P, E], BF16, tag=f"wv{k}", name=f"wv{k}")
                         for k in range(KT)]
                for k in range(KT):
                    nc.sync.dma_start(out=wq_sb[k][:],
                                      in_=wq[k * P:(k + 1) * P, :])
                    nc.sync.dma_start(out=wk_sb[k][:],
                                      in_=wk[k * P:(k + 1) * P, :])
                    nc.sync.dma_start(out=wv_sb[k][:],
                                      in_=wv[k * P:(k + 1) * P, :])

                qT = attnsb.tile([P, NTOK], BF16, tag="qT", name="qT")
                kTt = attnsb.tile([P, NTOK], BF16, tag="kT", name="kTt")
                v_sb = [attnsb.tile([P, E], BF16, tag=f"v{s}", name=f"v{s}")
                        for s in range(NTOK // P)]

                with tc.tile_pool(name="hstream", bufs=6) as hsp, \
                     tc.tile_pool(name="vtmp", bufs=2) as vtp, \
                     tc.tile_pool(name="qkvp", bufs=2, space="PSUM") as qkvp, \
                     tc.tile_pool(name="vtrp", bufs=2, space="PSUM") as vtrp:
                    for tch in range(NW):
                        psq = qkvp.tile([P, 512], F32, tag="psq", name="psq")
                        psk = qkvp.tile([P, 512], F32, tag="psk", name="psk")
                        psv = qkvp.tile([P, 512], F32, tag="psv", name="psv")
                        for kt in range(KT):
                            ht = hsp.tile([P, 512], BF16, tag="ht", name="ht")
                            nc.sync.dma_start(
                                out=ht[:],
                                in_=hT_all[tch * D + kt * P:
                                           tch * D + (kt + 1) * P, :])
                            first, last = kt == 0, kt == KT - 1
                            nc.tensor.matmul(psq[:], wq_sb[kt][:], ht[:],
                                             start=first, stop=last)
                            nc.tensor.matmul(psk[:], wk_sb[kt][:], ht[:],
                                             start=first, stop=last)
                            nc.tensor.matmul(psv[:], wv_sb[kt][:], ht[:],
                                             start=first, stop=last)
                        nc.scalar.copy(qT[:, tch * 512:(tch + 1) * 512], psq[:])
                        nc.scalar.copy(kTt[:, tch * 512:(tch + 1) * 512],
                                       psk[:])
                        vt = vtp.tile([P, 512], F32, tag="vt", name="vt")
                        nc.scalar.copy(vt[:], psv[:])
                        for j in range(4):
                            pv = vtrp.tile([P, P], F32, tag="pv", name="pv")
                            nc.tensor.transpose(pv[:], vt[:, j * P:(j + 1) * P],
                                                ident[:])
                            nc.vector.tensor_copy(v_sb[tch * 4 + j][:], pv[:])

                # -------- attention --------
                if upto < 3:
                    return
                masks = []
                for k in range(4):
                    m = attnsb.tile([P, 512], F32, tag=f"mask{k}",
                                    name=f"mask{k}")
                    nc.gpsimd.memset(m[:], 0.0)
                    nc.gpsimd.affine_select(
                        out=m[:], in_=m[:],
                        compare_op=mybir.AluOpType.is_ge,
                        fill=MASK_VAL, base=-128 * k,
                        pattern=[[1, 512]], channel_multiplier=-1,
                    )
                    masks.append(m)
                with tc.tile_pool(name="pt_pool", bufs=4) as ptp, \
                     tc.tile_pool(name="attno", bufs=2) as aop, \
                     tc.tile_pool(name="scp", bufs=2, space="PSUM") as scp, \
                     tc.tile_pool(name="lop", bufs=1, space="PSUM") as lop:
                    for b in range(B):
                        for tcl in range(T // 512):
                            tch = b * (T // 512) + tcl
                            l_psa = lop.tile([HD, 512], F32, tag="la", name="l_psa")
                            l_psb = lop.tile([HD, 512], F32, tag="lb", name="l_psb")
                            o_psa = lop.tile([HD, 512], F32, tag="oa", name="o_psa")
                            o_psb = lop.tile([HD, 512], F32, tag="ob", name="o_psb")
                            n_s = 4 * (tcl + 1)
                            for si in range(n_s):
                                sg = b * ST_B + si
                                sc_a = scp.tile([P, 512], F32, tag="sca",
                                                name="sc_a")
                                sc_b = scp.tile([P, 512], F32, tag="scb",
                                                name="sc_b")
                                nc.tensor.matmul(
                                    sc_a[:], kTt[0:HD, sg * P:(sg + 1) * P],
                                    qT[0:HD, tch * 512:(tch + 1) * 512],
                                    start=True, stop=True,
                                    tile_position=(0, 0))
                                nc.tensor.matmul(
                                    sc_b[:], kTt[HD:P, sg * P:(sg + 1) * P],
                                    qT[HD:P, tch * 512:(tch + 1) * 512],
                                    start=True, stop=True,
                                    tile_position=(64, 0))
                                if si // 4 == tcl:
                                    mk = masks[si % 4]
                                    nc.vector.tensor_add(out=sc_a[:],
                                                         in0=sc_a[:],
                                                         in1=mk[:])
                                    nc.vector.tensor_add(out=sc_b[:],
                                                         in0=sc_b[:],
                                                         in1=mk[:])
                                p_a = ptp.tile([P, 512], BF16, tag="pa",
                                               name="p_a")
                                p_b = ptp.tile([P, 512], BF16, tag="pb",
                                               name="p_b")
                                nc.scalar.activation(out=p_a[:], in_=sc_a[:],
                                                     func=AF.Exp, scale=SCALE)
                                nc.scalar.activation(out=p_b[:], in_=sc_b[:],
                                                     func=AF.Exp, scale=SCALE)
                                first, last = si == 0, si == n_s - 1
                                nc.tensor.matmul(l_psa[:], ones_r[:, 0:HD],
                                                 p_a[:], start=first, stop=last)
                                nc.tensor.matmul(l_psb[:], ones_r[:, 0:HD],
                                                 p_b[:], start=first, stop=last)
                                nc.tensor.matmul(o_psa[:],
                                                 v_sb[sg][:, 0:HD], p_a[:],
                                                 start=first, stop=last)
                                nc.tensor.matmul(o_psb[:],
                                                 v_sb[sg][:, HD:E], p_b[:],
                                                 start=first, stop=last)
                            linv = aop.tile([P, 512], F32, tag="linv",
                                            name="linv")
                            nc.vector.reciprocal(out=linv[0:HD, :], in_=l_psa[:])
                            nc.vector.reciprocal(out=linv[HD:P, :], in_=l_psb[:])
                            o_n = aop.tile([P, 512], F32R, tag="on", name="o_n")
                            nc.vector.tensor_mul(out=o_n[0:HD, :], in0=o_psa[:],
                                                 in1=linv[0:HD, :])
                            nc.vector.tensor_mul(out=o_n[HD:P, :], in0=o_psb[:],
                                                 in1=linv[HD:P, :])
                            nc.sync.dma_start(
                                out=a2a_in[tch * P:(tch + 1) * P, :],
                                in_=o_n[:])

                if stub_cc:
                    nc.sync.dma_start(out=a2a_out[:, :], in_=a2a_in[:, :])
                else:
                    nc.gpsimd.collective_compute(
                        "AllToAll", mybir.AluOpType.bypass,
                        replica_groups=[list(range(N_CORES))],
                        ins=[a2a_in.opt()], outs=[a2a_out.opt()],
                    )
            # attnsb closed: qT/kT/v/wqkv SBUF freed

            # -------- output projection + residual --------
            if upto < 4:
                return
            with tc.tile_pool(name="wos", bufs=3) as wos, \
                 tc.tile_pool(name="aos", bufs=3) as aos, \
                 tc.tile_pool(name="wop", bufs=1, space="PSUM") as wop:
                pso = [wop.tile([P, 512], F32, tag=f"wo{i}", name=f"wo{i}")
                       for i in range(8)]
                for kt in range(KT):
                    ao = aos.tile([P, S], F32R, tag="ao", name="ao")
                    nc.sync.dma_start(out=ao[:],
                                      in_=a2a_out[kt * P:(kt + 1) * P, :])
                    wot = wos.tile([P, D], F32R, tag="wot", name="wot")
                    nc.sync.dma_start(out=wot[:],
                                      in_=wo[kt * P:(kt + 1) * P, :])
                    first, last = kt == 0, kt == KT - 1
                    for tt in range(TT):
                        for dc in range(2):
                            nc.tensor.matmul(
                                pso[tt * 2 + dc][:],
                                ao[:, tt * P:(tt + 1) * P],
                                wot[:, dc * 512:(dc + 1) * 512],
                                start=first, stop=last)
                for tt in range(TT):
                    for dc in range(2):
                        sl = slice(dc * 512, (dc + 1) * 512)
                        nc.vector.tensor_add(out=r1[tt][:, sl],
                                             in0=pso[tt * 2 + dc][:],
                                             in1=x_sb[tt][:, sl])
                        nc.vector.tensor_add(out=r1[tt][:, sl],
                                             in0=r1[tt][:, sl],
                                             in1=bo_bc[:, sl])

            # -------- LN2 + transpose --------
            if upto < 5:
                return
            with tc.tile_pool(name="h2p", bufs=1) as h2p:
                h2 = layernorm_tiles(r1, h2p, "h2")
                with tc.tile_pool(name="tr2p", bufs=3, space="PSUM") as tr2p:
                    for i in range(TT):
                        for kt in range(KT):
                            pt2 = tr2p.tile([P, P], F32, tag="tr2", name="pt2")
                            nc.tensor.transpose(
                                pt2[:], h2[i][:, kt * P:(kt + 1) * P], ident[:])
                            nc.vector.tensor_scalar(
                                out=h2T[kt][:, i * P:(i + 1) * P], in0=pt2[:],
                                scalar1=g2_s[:, kt:kt + 1],
                                scalar2=b2l_s[:, kt:kt + 1],
                                op0=mybir.AluOpType.mult,
                                op1=mybir.AluOpType.add,
                            )

            # -------- FFN --------
            if upto < 6:
                return
            with tc.tile_pool(name="ff1sb", bufs=1) as ff1sb:
                ff1 = [ff1sb.tile([P, S], F32R, tag=f"ff1_{k}",
                                  name=f"ff1_{k}") for k in range(FT)]
                with tc.tile_pool(name="w1s", bufs=1) as w1s, \
                     tc.tile_pool(name="ff1p", bufs=3, space="PSUM") as ff1p:
                    FH = F // 2
                    for half in range(2):
                        w1h = [w1s.tile([P, FH], F32R, tag=f"w1h{k}",
                                        name=f"w1h{k}") for k in range(KT)]
                        for k in range(KT):
                            nc.sync.dma_start(
                                out=w1h[k][:],
                                in_=w1[k * P:(k + 1) * P,
                                       half * FH:(half + 1) * FH])
                        for fl in range(FH // P):
                            ft = half * (FH // P) + fl
                            ps = ff1p.tile([P, S], F32, tag="ff1", name="ps")
                            for kt in range(KT):
                                nc.tensor.matmul(
                                    ps[:], w1h[kt][:, fl * P:(fl + 1) * P],
                                    h2T[kt][:],
                                    start=(kt == 0), stop=(kt == KT - 1))
                            nc.scalar.activation(out=ff1[ft][:], in_=ps[:],
                                                 func=AF.Relu,
                                                 bias=b1_s[:, ft:ft + 1])

                if upto < 7:
                    return
                with tc.tile_pool(name="w2s", bufs=4) as w2s, \
                     tc.tile_pool(name="outp", bufs=4) as outp, \
                     tc.tile_pool(name="ff2p", bufs=1, space="PSUM") as ff2p:
                    ps2 = [ff2p.tile([P, 512], F32, tag=f"ff2_{i}",
                                     name=f"ff2_{i}") for i in range(8)]
                    for kt in range(FT):
                        w2t = w2s.tile([P, D], F32R, tag="w2t", name="w2t")
                        nc.sync.dma_start(out=w2t[:],
                                          in_=w2[kt * P:(kt + 1) * P, :])
                        first, last = kt == 0, kt == FT - 1
                        for tt in range(TT):
                            for dc in range(2):
                                nc.tensor.matmul(
                                    ps2[tt * 2 + dc][:],
                                    ff1[kt][:, tt * P:(tt + 1) * P],
                                    w2t[:, dc * 512:(dc + 1) * 512],
                                    start=first, stop=last)
                    for tt in range(TT):
                        for dc in range(2):
                            sl = slice(dc * 512, (dc + 1) * 512)
                            ot = outp.tile([P, 512], F32, tag="ot", name="ot")
                            nc.vector.tensor_add(out=ot[:],
                                                 in0=ps2[tt * 2 + dc][:],
                                                 in1=r1[tt][:, sl])
                            nc.vector.tensor_add(out=ot[:], in0=ot[:],
                                                 in1=b2_bc[:, sl])
                            nc.sync.dma_start(out=y[tt * P:(tt + 1) * P, sl],
                                              in_=ot[:])


def _shard_inputs(inputs):
    x = np.ascontiguousarray(np.asarray(inputs["x"], np.float32).reshape(NTOK, D))
    Wq = np.asarray(inputs["Wq"], np.float32)
    Wk = np.asarray(inputs["Wk"], np.float32)
    Wv = np.asarray(inputs["Wv"], np.float32)
    com = dict(
        wo=np.ascontiguousarray(np.asarray(inputs["Wo"], np.float32)),
        w1=np.ascontiguousarray(np.asarray(inputs["W1"], np.float32)),
        w2=np.ascontiguousarray(np.asarray(inputs["W2"], np.float32)),
        bo=np.asarray(inputs["bo"], np.float32),
        b1=np.asarray(inputs["b1"], np.float32),
        b2=np.asarray(inputs["b2"], np.float32),
        ln1_g=np.asarray(inputs["ln1_g"], np.float32),
        ln1_b=np.asarray(inputs["ln1_b"], np.float32),
        ln2_g=np.asarray(inputs["ln2_g"], np.float32),
        ln2_b=np.asarray(inputs["ln2_b"], np.float32),
    )
    maps = []
    for c in range(N_CORES):
        hs = slice(HPC * c, HPC * (c + 1))
        m = dict(com)
        m["x"] = x[c * S:(c + 1) * S]
        import ml_dtypes
        bf = ml_dtypes.bfloat16
        m["wq"] = np.ascontiguousarray(
            Wq[hs].transpose(1, 0, 2).reshape(D, E).astype(bf))
        m["wk"] = np.ascontiguousarray(
            Wk[hs].transpose(1, 0, 2).reshape(D, E).astype(bf))
        m["wv"] = np.ascontiguousarray(
            Wv[hs].transpose(1, 0, 2).reshape(D, E).astype(bf))
        maps.append(m)
    return maps


def _get_nc():
    if "nc" not in _CACHE:
        _CACHE["nc"] = _build()
    return _CACHE["nc"]


def _run(in_maps):
    from concourse.bass_utils import run_bass_kernel_spmd
    nc = _get_nc()
    res = run_bass_kernel_spmd(nc, in_maps, core_ids=list(range(N_CORES)))
    return res.results


def kernel(**inputs):
    in_maps = _shard_inputs(inputs)
    results = _run(in_maps)
    out = np.concatenate([results[c]["y"] for c in range(N_CORES)], axis=0)
    return out.reshape(B, T, D)


# revision 16
# speedup vs baseline: 8881.9463x; 1.6722x over previous
"""Distributed Trainium2 kernel for a dense transformer block.

Sharding (8 cores, one chip):
  - LN1/LN2 + FFN: sequence-parallel (each core owns 512 of the 4096 tokens).
  - Attention: head-parallel (each core owns 2 of the 16 heads).
  - Collectives: AllGather of ln1(x)^T (feature-major), AllToAll of per-head
    attention outputs back to token shards.  No AllReduce needed.
  - Matmuls run in float32r (full-rate reduced-precision fp32) with fp32
    accumulation in PSUM.
"""

import sys

sys.path.insert(0, "/opt/trn_rl_repo")

import numpy as np

import concourse.bacc as bacc
import concourse.bass as bass
import concourse.tile as tile
from concourse import mybir
from concourse.masks import make_identity

F32 = mybir.dt.float32
F32R = mybir.dt.float32r
BF16 = mybir.dt.bfloat16
AF = mybir.ActivationFunctionType

N_CORES = 8
B, T, D, H = 2, 2048, 1024, 16
HD = D // H            # 64
NTOK = B * T           # 4096
S = NTOK // N_CORES    # 512 tokens per core
HPC = H // N_CORES     # 2 heads per core
E = HPC * HD           # 128 head-dim columns per core
F = 4 * D              # 4096 ffn hidden
EPS = 1e-5
SCALE = float(D) ** -0.5
MASK_VAL = -30000.0
P = 128

KT = D // P            # 8 feature tiles
TT = S // P            # 4 token tiles in the shard
NW = N_CORES           # 8 global 512-token windows
ST_B = T // P          # 16 s-tiles per batch
FT = F // P            # 32 ffn-hidden tiles

_CACHE = {}


def _build(n_chain=1, stub_cc=False, upto=9):
    nc = bacc.Bacc("TRN2", target_bir_lowering=False, debug=False,
                   num_devices=N_CORES)

    x = nc.dram_tensor("x", [S, D], F32, kind="ExternalInput")
    wq = nc.dram_tensor("wq", [D, E], BF16, kind="ExternalInput")
    wk = nc.dram_tensor("wk", [D, E], BF16, kind="ExternalInput")
    wv = nc.dram_tensor("wv", [D, E], BF16, kind="ExternalInput")
    wo = nc.dram_tensor("wo", [D, D], F32R, kind="ExternalInput")
    w1 = nc.dram_tensor("w1", [D, F], F32R, kind="ExternalInput")
    w2 = nc.dram_tensor("w2", [F, D], F32R, kind="ExternalInput")
    bo = nc.dram_tensor("bo", [D], F32, kind="ExternalInput")
    b1 = nc.dram_tensor("b1", [F], F32, kind="ExternalInput")
    b2 = nc.dram_tensor("b2", [D], F32, kind="ExternalInput")
    ln1_g = nc.dram_tensor("ln1_g", [D], F32, kind="ExternalInput")
    ln1_b = nc.dram_tensor("ln1_b", [D], F32, kind="ExternalInput")
    ln2_g = nc.dram_tensor("ln2_g", [D], F32, kind="ExternalInput")
    ln2_b = nc.dram_tensor("ln2_b", [D], F32, kind="ExternalInput")
    y = nc.dram_tensor("y", [S, D], F32, kind="ExternalOutput")
    global _W
    _W = dict(wq=wq, wk=wk, wv=wv, wo=wo, w1=w1, w2=w2, bo=bo, b1=b1, b2=b2,
              ln1_g=ln1_g, ln1_b=ln1_b, ln2_g=ln2_g, ln2_b=ln2_b)

    with tile.TileContext(nc) as tc:
      with tc.tile_pool(name="dram0", bufs=1, space="DRAM") as dram0:
        chain_bufs = [dram0.tile([S, D], F32, tag=f"chain{i}", name=f"chain{i}")
                      for i in range(n_chain - 1)]
        for _ci in range(n_chain):
            x_cur = x if _ci == 0 else chain_bufs[_ci - 1]
            y_cur = y if _ci == n_chain - 1 else chain_bufs[_ci]
            _emit_body(nc, tc, x_cur, y_cur, _ci, stub_cc, upto)

    nc.compile()
    return nc


def _emit_body(nc, tc, x, y, ci, stub_cc=False, upto=9):
    wq, wk, wv, wo = _W["wq"], _W["wk"], _W["wv"], _W["wo"]
    w1, w2, bo, b1, b2 = _W["w1"], _W["w2"], _W["bo"], _W["b1"], _W["b2"]
    ln1_g, ln1_b = _W["ln1_g"], _W["ln1_b"]
    ln2_g, ln2_b = _W["ln2_g"], _W["ln2_b"]
    with 1 == 1 and tc.tile_pool(name=f"body{ci}", bufs=1) as _unused:
        with tc.tile_pool(name="dram", bufs=1, space="DRAM") as dram, \
             tc.tile_pool(name="const", bufs=1) as const, \
             tc.tile_pool(name="persist", bufs=1) as persist:

            hT_sh = dram.tile([D, S], BF16, tag="hT_sh", name="hT_sh")
            hT_all = dram.tile([N_CORES * D, S], BF16, tag="hT_all",
                               name="hT_all", addr_space="Shared")
            a2a_in = dram.tile([NW * P, S], F32R, tag="a2a_in", name="a2a_in")
            a2a_out = dram.tile([NW * P, S], F32R, tag="a2a_out",
                                name="a2a_out")

            # ---- constants ----
            ident = const.tile([P, P], F32, tag="ident", name="ident")
            make_identity(nc, ident)

            ones_r = const.tile([P, HD], BF16, tag="ones_r", name="ones_r")
            nc.vector.memset(ones_r[:], 1.0)

            eps_t = const.tile([P, 1], F32, tag="eps", name="eps_t")
            nc.vector.memset(eps_t[:], EPS)

            # ln params, feature-major [128, KT]
            g1_s = const.tile([P, KT], F32, tag="g1", name="g1_s")
            b1l_s = const.tile([P, KT], F32, tag="b1l", name="b1l_s")
            g2_s = const.tile([P, KT], F32, tag="g2", name="g2_s")
            b2l_s = const.tile([P, KT], F32, tag="b2l", name="b2l_s")
            nc.sync.dma_start(out=g1_s[:],
                              in_=ln1_g.ap().rearrange("(k p) -> p k", p=P))
            nc.sync.dma_start(out=b1l_s[:],
                              in_=ln1_b.ap().rearrange("(k p) -> p k", p=P))
            nc.sync.dma_start(out=g2_s[:],
                              in_=ln2_g.ap().rearrange("(k p) -> p k", p=P))
            nc.sync.dma_start(out=b2l_s[:],
                              in_=ln2_b.ap().rearrange("(k p) -> p k", p=P))

            # b1 (ffn bias), feature-major [128, FT]
            b1_s = const.tile([P, FT], F32, tag="b1s", name="b1_s")
            nc.sync.dma_start(out=b1_s[:],
                              in_=b1.ap().rearrange("(k p) -> p k", p=P))

            # bo, b2 broadcast across partitions [128, D]
            bo_bc = const.tile([P, D], F32, tag="bo_bc", name="bo_bc")
            b2_bc = const.tile([P, D], F32, tag="b2_bc", name="b2_bc")
            nc.sync.dma_start(out=bo_bc[:], in_=bo.ap().partition_broadcast(P))
            nc.sync.dma_start(out=b2_bc[:], in_=b2.ap().partition_broadcast(P))

            # persistent across most of the kernel: x shard, r1, h2T
            x_sb = [persist.tile([P, D], F32, tag=f"x{i}", name=f"x{i}")
                    for i in range(TT)]
            for i in range(TT):
                nc.sync.dma_start(out=x_sb[i][:], in_=x[i * P:(i + 1) * P, :])
            r1 = [persist.tile([P, D], F32, tag=f"r1_{i}", name=f"r1_{i}")
                  for i in range(TT)]
            h2T = [persist.tile([P, S], F32R, tag=f"h2T{k}", name=f"h2T{k}")
                   for k in range(KT)]

            def layernorm_tiles(src_tiles, pool, out_name):
                """LN over the free axis -> normalized [t,d] f32 tiles.
                gamma/beta are folded in at transpose-evict time."""
                out = []
                with tc.tile_pool(name=f"ln_{out_name}", bufs=2) as lnp:
                    for i, xt in enumerate(src_tiles):
                        st = lnp.tile([P, 2, 6], F32, tag="stats", name="st")
                        xr = xt[:].rearrange("p (s f) -> p s f", s=2)
                        for sg in range(2):
                            nc.vector.bn_stats(out=st[:, sg, :], in_=xr[:, sg, :])
                        mv = lnp.tile([P, 2], F32, tag="mv", name="mv")
                        nc.vector.bn_aggr(out=mv[:], in_=st[:])
                        rstd = lnp.tile([P, 1], F32, tag="rstd", name="rstd")
                        nc.scalar.activation(out=rstd[:], in_=mv[:, 1:2],
                                             func=AF.Sqrt, bias=eps_t[:],
                                             scale=1.0)
                        nc.vector.reciprocal(out=rstd[:], in_=rstd[:])
                        o = pool.tile([P, D], F32, tag=f"{out_name}{i}",
                                      name=f"{out_name}{i}")
                        nc.vector.tensor_scalar(
                            out=o[:], in0=xt[:],
                            scalar1=mv[:, 0:1], scalar2=rstd[:],
                            op0=mybir.AluOpType.subtract,
                            op1=mybir.AluOpType.mult,
                        )
                        out.append(o)
                return out

            # ============ attention super-phase (scoped SBUF) ============
            with tc.tile_pool(name="attnsb", bufs=1) as attnsb:
                # -------- LN1 + transpose + AllGather --------
                with tc.tile_pool(name="xlnp", bufs=1) as xlnp:
                    xln = layernorm_tiles(x_sb, xlnp, "xln")
                    with tc.tile_pool(name="tr1", bufs=4) as trp, \
                         tc.tile_pool(name="tr1p", bufs=4, space="PSUM") as trpp:
                        for kt in range(KT):
                            hb = trp.tile([P, S], BF16, tag="hb", name="hb")
                            for i in range(TT):
                                pt = trpp.tile([P, P], F32, tag="tr", name="pt")
                                nc.tensor.transpose(
                                    pt[:], xln[i][:, kt * P:(kt + 1) * P],
                                    ident[:])
                                nc.vector.tensor_scalar(
                                    out=hb[:, i * P:(i + 1) * P], in0=pt[:],
                                    scalar1=g1_s[:, kt:kt + 1],
                                    scalar2=b1l_s[:, kt:kt + 1],
                                    op0=mybir.AluOpType.mult,
                                    op1=mybir.AluOpType.add,
                                )
                            nc.sync.dma_start(
                                out=hT_sh[kt * P:(kt + 1) * P, :], in_=hb[:])

                if stub_cc:
                    nc.sync.dma_start(out=hT_all[0:D, :], in_=hT_sh[:, :])
                else:
                    nc.gpsimd.collective_compute(
                        "AllGather", mybir.AluOpType.bypass,
                        replica_groups=[list(range(N_CORES))],
                        ins=[hT_sh.opt()], outs=[hT_all.opt()],
                    )

                # -------- QKV projections --------
                if upto < 2:
                    return
                wq_sb = [attnsb.tile([P, E], BF16, tag=f"wq{k}", name=f"wq{k}")
                         for k in range(KT)]
                wk_sb = [attnsb.tile([P, E], BF16, tag=f"wk{k}", name=f"wk{k}")
                         for k in range(KT)]
                wv_sb = [attnsb.tile([P, E], BF16, tag=f"wv{k}", name=f"wv{k}")
                         for k in range(KT)]
                for k in range(KT):
                    nc.sync.dma_start(out=wq_sb[k][:],
                                      in_=wq[k * P:(k + 1) * P, :])
                    nc.sync.dma_start(out=wk_sb[k][:],
                                      in_=wk[k * P:(k + 1) * P, :])
                    nc.sync.dma_start(out=wv_sb[k][:],
                                      in_=wv[k * P:(k + 1) * P, :])

                qT = attnsb.tile([P, NTOK], BF16, tag="qT", name="qT")
                kTt = attnsb.tile([P, NTOK], BF16, tag="kT", name="kTt")
                v_sb = [attnsb.tile([P, E], BF16, tag=f"v{s}", name=f"v{s}")
                        for s in range(NTOK // P)]

                with tc.tile_pool(name="hstream", bufs=6) as hsp, \
                     tc.tile_pool(name="vtmp", bufs=2) as vtp, \
                     tc.tile_pool(name="qkvp", bufs=2, space="PSUM") as qkvp, \
                     tc.tile_pool(name="vtrp", bufs=2, space="PSUM") as vtrp:
                    for tch in range(NW):
                        psq = qkvp.tile([P, 512], F32, tag="psq", name="psq")
                        psk = qkvp.tile([P, 512], F32, tag="psk", name="psk")
                        psv = qkvp.tile([P, 512], F32, tag="psv", name="psv")
                        for kt in range(KT):
                            ht = hsp.tile([P, 512], BF16, tag="ht", name="ht")
                            nc.sync.dma_start(
                                out=ht[:],
                                in_=hT_all[tch * D + kt * P:
                                           tch * D + (kt + 1) * P, :])
                            first, last = kt == 0, kt == KT - 1
                            nc.tensor.matmul(psq[:], wq_sb[kt][:], ht[:],
                                             start=first, stop=last)
                            nc.tensor.matmul(psk[:], wk_sb[kt][:], ht[:],
                                             start=first, stop=last)
                            nc.tensor.matmul(psv[:], wv_sb[kt][:], ht[:],
                                             start=first, stop=last)
                        nc.vector.tensor_copy(qT[:, tch * 512:(tch + 1) * 512],
                                              psq[:])
                        nc.vector.tensor_copy(kTt[:, tch * 512:(tch + 1) * 512],
                                              psk[:])
                        vt = vtp.tile([P, 512], F32, tag="vt", name="vt")
                        nc.scalar.copy(vt[:], psv[:])
                        for j in range(4):
                            pv = vtrp.tile([P, P], F32, tag="pv", name="pv")
                            nc.tensor.transpose(pv[:], vt[:, j * P:(j + 1) * P],
                                                ident[:])
                            nc.vector.tensor_copy(v_sb[tch * 4 + j][:], pv[:])

                # -------- attention --------
                if upto < 3:
                    return
                masks = []
                for k in range(4):
                    m = attnsb.tile([P, 512], F32, tag=f"mask{k}",
                                    name=f"mask{k}")
                    nc.gpsimd.memset(m[:], 0.0)
                    nc.gpsimd.affine_select(
                        out=m[:], in_=m[:],
                        compare_op=mybir.AluOpType.is_ge,
                        fill=MASK_VAL, base=-128 * k,
                        pattern=[[1, 512]], channel_multiplier=-1,
                    )
                    masks.append(m)
                with tc.tile_pool(name="pt_pool", bufs=4) as ptp, \
                     tc.tile_pool(name="attno", bufs=2) as aop, \
                     tc.tile_pool(name="scp", bufs=2, space="PSUM") as scp, \
                     tc.tile_pool(name="lop", bufs=1, space="PSUM") as lop:
                    for b in range(B):
                        for tcl in range(T // 512):
                            tch = b * (T // 512) + tcl
                            l_psa = lop.tile([HD, 512], F32, tag="la", name="l_psa")
                            l_psb = lop.tile([HD, 512], F32, tag="lb", name="l_psb")
                            o_psa = lop.tile([HD, 512], F32, tag="oa", name="o_psa")
                            o_psb = lop.tile([HD, 512], F32, tag="ob", name="o_psb")
                            n_s = 4 * (tcl + 1)
                            for si in range(n_s):
                                sg = b * ST_B + si
                                sc_a = scp.tile([P, 512], F32, tag="sca",
                                                name="sc_a")
                                sc_b = scp.tile([P, 512], F32, tag="scb",
                                                name="sc_b")
                                nc.tensor.matmul(
                                    sc_a[:], kTt[0:HD, sg * P:(sg + 1) * P],
                                    qT[0:HD, tch * 512:(tch + 1) * 512],
                                    start=True, stop=True,
                                    tile_position=(0, 0))
                                nc.tensor.matmul(
                                    sc_b[:], kTt[HD:P, sg * P:(sg + 1) * P],
                                    qT[HD:P, tch * 512:(tch + 1) * 512],
                                    start=True, stop=True,
                                    tile_position=(64, 0))
                                if si // 4 == tcl:
                                    mk = masks[si % 4]
                                    nc.vector.tensor_add(out=sc_a[:],
                                                         in0=sc_a[:],
                                                         in1=mk[:])
                                    nc.vector.tensor_add(out=sc_b[:],
                                                         in0=sc_b[:],
                                                         in1=mk[:])
                                p_a = ptp.tile([P, 512], BF16, tag="pa",
                                               name="p_a")
                                p_b = ptp.tile([P, 512], BF16, tag="pb",
                                               name="p_b")
                                nc.scalar.activation(out=p_a[:], in_=sc_a[:],
                                                     func=AF.Exp, scale=SCALE)
                                nc.scalar.activation(out=p_b[:], in_=sc_b[:],
                                                     func=AF.Exp, scale=SCALE)
                                first, last = si == 0, si == n_s - 1
                                nc.tensor.matmul(l_psa[:], ones_r[:, 0:HD],
                                                 p_a[:], start=first, stop=last)
                                nc.tensor.matmul(l_psb[:], ones_r[:, 0:HD],
                                                 p_b[:], start=first, stop=last)
                                nc.tensor.matmul(o_psa[:],
                                                 v_sb[sg][:, 0:HD], p_a[:],
                                                 start=first, stop=last)
                                nc.tensor.matmul(o_psb[:],
                                                 v_sb[sg][:, HD:E], p_b[:],
                                                 start=first, stop=last)
                            linv = aop.tile([P, 512], F32, tag="linv",
                                            name="linv")
                            nc.vector.reciprocal(out=linv[0:HD, :], in_=l_psa[:])
                            nc.vector.reciprocal(out=linv[HD:P, :], in_=l_psb[:])
                            o_n = aop.tile([P, 512], F32R, tag="on", name="o_n")
                            nc.vector.tensor_mul(out=o_n[0:HD, :], in0=o_psa[:],
                                                 in1=linv[0:HD, :])
                            nc.vector.tensor_mul(out=o_n[HD:P, :], in0=o_psb[:],
                                                 in1=linv[HD:P, :])
                            nc.sync.dma_start(
                                out=a2a_in[tch * P:(tch + 1) * P, :],
                                in_=o_n[:])

                if stub_cc:
                    nc.sync.dma_start(out=a2a_out[:, :], in_=a2a_in[:, :])
                else:
                    nc.gpsimd.collective_compute(
                        "AllToAll", mybir.AluOpType.bypass,
                        replica_groups=[list(range(N_CORES))],
                        ins=[a2a_in.opt()], outs=[a2a_out.opt()],
                    )
            # attnsb closed: qT/kT/v/wqkv SBUF freed

            # -------- output projection + residual --------
            if upto < 4:
                return
            with tc.tile_pool(name="wos", bufs=3) as wos, \
                 tc.tile_pool(name="aos", bufs=3) as aos, \
                 tc.tile_pool(name="wop", bufs=1, space="PSUM") as wop:
                pso = [wop.tile([P, 512], F32, tag=f"wo{i}", name=f"wo{i}")
                       for i in range(8)]
                for kt in range(KT):
                    ao = aos.tile([P, S], F32R, tag="ao", name="ao")
                    nc.sync.dma_start(out=ao[:],
                                      in_=a2a_out[kt * P:(kt + 1) * P, :])
                    wot = wos.tile([P, D], F32R, tag="wot", name="wot")
                    nc.sync.dma_start(out=wot[:],
                                      in_=wo[kt * P:(kt + 1) * P, :])
                    first, last = kt == 0, kt == KT - 1
                    for tt in range(TT):
                        for dc in range(2):
                            nc.tensor.matmul(
                                pso[tt * 2 + dc][:],
                                ao[:, tt * P:(tt + 1) * P],
                                wot[:, dc * 512:(dc + 1) * 512],
                                start=first, stop=last)
                for tt in range(TT):
                    for dc in range(2):
                        sl = slice(dc * 512, (dc + 1) * 512)
                        nc.vector.tensor_add(out=r1[tt][:, sl],
                                             in0=pso[tt * 2 + dc][:],
                                             in1=x_sb[tt][:, sl])
                        nc.vector.tensor_add(out=r1[tt][:, sl],
                                             in0=r1[tt][:, sl],
                                             in1=bo_bc[:, sl])

            # -------- LN2 + transpose --------
            if upto < 5:
                return
            with tc.tile_pool(name="h2p", bufs=1) as h2p:
                h2 = layernorm_tiles(r1, h2p, "h2")
                with tc.tile_pool(name="tr2p", bufs=3, space="PSUM") as tr2p:
                    for i in range(TT):
                        for kt in range(KT):
                            pt2 = tr2p.tile([P, P], F32, tag="tr2", name="pt2")
                            nc.tensor.transpose(
                                pt2[:], h2[i][:, kt * P:(kt + 1) * P], ident[:])
                            nc.vector.tensor_scalar(
                                out=h2T[kt][:, i * P:(i + 1) * P], in0=pt2[:],
                                scalar1=g2_s[:, kt:kt + 1],
                                scalar2=b2l_s[:, kt:kt + 1],
                                op0=mybir.AluOpType.mult,
                                op1=mybir.AluOpType.add,
                            )

            # -------- FFN --------
            if upto < 6:
                return
            with tc.tile_pool(name="ff1sb", bufs=1) as ff1sb:
                ff1 = [ff1sb.tile([P, S], F32R, tag=f"ff1_{k}",
                                  name=f"ff1_{k}") for k in range(FT)]
                with tc.tile_pool(name="w1s", bufs=1) as w1s, \
                     tc.tile_pool(name="ff1p", bufs=3, space="PSUM") as ff1p:
                    FH = F // 2
                    for half in range(2):
                        w1h = [w1s.tile([P, FH], F32R, tag=f"w1h{k}",
                                        name=f"w1h{k}") for k in range(KT)]
                        for k in range(KT):
                            nc.sync.dma_start(
                                out=w1h[k][:],
                                in_=w1[k * P:(k + 1) * P,
                                       half * FH:(half + 1) * FH])
                        for fl in range(FH // P):
                            ft = half * (FH // P) + fl
                            ps = ff1p.tile([P, S], F32, tag="ff1", name="ps")
                            for kt in range(KT):
                                nc.tensor.matmul(
                                    ps[:], w1h[kt][:, fl * P:(fl + 1) * P],
                                    h2T[kt][:],
                                    start=(kt == 0), stop=(kt == KT - 1))
                            nc.scalar.activation(out=ff1[ft][:], in_=ps[:],
                                                 func=AF.Relu,
                                                 bias=b1_s[:, ft:ft + 1])

                if upto < 7:
                    return
                with tc.tile_pool(name="w2s", bufs=4) as w2s, \
                     tc.tile_pool(name="outp", bufs=4) as outp, \
                     tc.tile_pool(name="ff2p", bufs=1, space="PSUM") as ff2p:
                    ps2 = [ff2p.tile([P, 512], F32, tag=f"ff2_{i}",
                                     name=f"ff2_{i}") for i in range(8)]
                    for kt in range(FT):
                        w2t = w2s.tile([P, D], F32R, tag="w2t", name="w2t")
                        nc.sync.dma_start(out=w2t[:],
                                          in_=w2[kt * P:(kt + 1) * P, :])
                        first, last = kt == 0, kt == FT - 1
                        for tt in range(TT):
                            for dc in range(2):
                                nc.tensor.matmul(
                                    ps2[tt * 2 + dc][:],
                                    ff1[kt][:, tt * P:(tt + 1) * P],
                                    w2t[:, dc * 512:(dc + 1) * 512],
                                    start=first, stop=last)
                    for tt in range(TT):
                        for dc in range(2):
                            sl = slice(dc * 512, (dc + 1) * 512)
                            ot = outp.tile([P, 512], F32, tag="ot", name="ot")
                            nc.vector.tensor_add(out=ot[:],
                                                 in0=ps2[tt * 2 + dc][:],
                                                 in1=r1[tt][:, sl])
                            nc.vector.tensor_add(out=ot[:], in0=ot[:],
                                                 in1=b2_bc[:, sl])
                            nc.sync.dma_start(out=y[tt * P:(tt + 1) * P, sl],
                                              in_=ot[:])


def _shard_inputs(inputs):
    x = np.ascontiguousarray(np.asarray(inputs["x"], np.float32).reshape(NTOK, D))
    Wq = np.asarray(inputs["Wq"], np.float32)
    Wk = np.asarray(inputs["Wk"], np.float32)
    Wv = np.asarray(inputs["Wv"], np.float32)
    com = dict(
        wo=np.ascontiguousarray(np.asarray(inputs["Wo"], np.float32)),
        w1=np.ascontiguousarray(np.asarray(inputs["W1"], np.float32)),
        w2=np.ascontiguousarray(np.asarray(inputs["W2"], np.float32)),
        bo=np.asarray(inputs["bo"], np.float32),
        b1=np.asarray(inputs["b1"], np.float32),
        b2=np.asarray(inputs["b2"], np.float32),
        ln1_g=np.asarray(inputs["ln1_g"], np.float32),
        ln1_b=np.asarray(inputs["ln1_b"], np.float32),
        ln2_g=np.asarray(inputs["ln2_g"], np.float32),
        ln2_b=np.asarray(inputs["ln2_b"], np.float32),
    )
    maps = []
    for c in range(N_CORES):
        hs = slice(HPC * c, HPC * (c + 1))
        m = dict(com)
        m["x"] = x[c * S:(c + 1) * S]
        import ml_dtypes
        bf = ml_dtypes.bfloat16
        m["wq"] = np.ascontiguousarray(
            Wq[hs].transpose(1, 0, 2).reshape(D, E).astype(bf))
        m["wk"] = np.ascontiguousarray(
            Wk[hs].transpose(1, 0, 2).reshape(D, E).astype(bf))
        m["wv"] = np.ascontiguousarray(
            Wv[hs].transpose(1, 0, 2).reshape(D, E).astype(bf))
        maps.append(m)
    return maps


def _get_nc():
    if "nc" not in _CACHE:
        _CACHE["nc"] = _build()
    return _CACHE["nc"]


def _run(in_maps):
    from concourse.bass_utils import run_bass_kernel_spmd
    nc = _get_nc()
    res = run_bass_kernel_spmd(nc, in_maps, core_ids=list(range(N_CORES)))
    return res.results


def kernel(**inputs):
    in_maps = _shard_inputs(inputs)
    results = _run(in_maps)
    out = np.concatenate([results[c]["y"] for c in range(N_CORES)], axis=0)
    return out.reshape(B, T, D)
